# revision 1
# baseline (speedup 1.0000x reference)
"""MinamoTopoModel GAT kernel: host preprocessing + Bass builder.

Self-contained logic module; kernel.py inlines/imports this during dev.
Design (per 8-core SPMD, dst-sharded):
  L1: cnt-histogram trick (host) -> per-group matmuls, no edge gathers.
  L2/L3: per-tile (128-edge) indirect DMA gathers of node records +
         S-matrix (iota-compare) PSUM scatter matmuls, segment softmax
         without max-subtraction, self-loops handled per-group directly.
  Two AllGathers publish per-shard node records between layers.
  Graph pooling -> per-core [50,17] partials; final FC on host.
"""
import numpy as np
import concourse.bacc as bacc
import concourse.bass as bass
import concourse.mybir as mybir
import concourse.tile as tile

F32 = mybir.dt.float32
I32 = mybir.dt.int32
AX = mybir.AxisListType
ALU = mybir.AluOpType
ACT = mybir.ActivationFunctionType
EPS = 1e-5


def host_prep(inputs, N, E, G, NC, TILE=32, EMB=16):
    H1, C1, H2, C2, H3, C3 = 8, 64, 4, 128, 1, 16
    x = np.asarray(inputs['x']).astype(np.int64)
    ei = np.asarray(inputs['edge_index']).astype(np.int64)
    batch = np.asarray(inputs['batch']).astype(np.int64)
    emb = np.asarray(inputs['emb'], np.float32)
    W1 = np.asarray(inputs['W1'], np.float32)
    as1 = np.asarray(inputs['a_src1'], np.float32); ad1 = np.asarray(inputs['a_dst1'], np.float32)
    b1 = np.asarray(inputs['b1'], np.float32)
    g1 = np.asarray(inputs['g1'], np.float32); be1 = np.asarray(inputs['be1'], np.float32)
    W2 = np.asarray(inputs['W2'], np.float32)
    as2 = np.asarray(inputs['a_src2'], np.float32); ad2 = np.asarray(inputs['a_dst2'], np.float32)
    b2 = np.asarray(inputs['b2'], np.float32)
    g2 = np.asarray(inputs['g2'], np.float32); be2 = np.asarray(inputs['be2'], np.float32)
    W3 = np.asarray(inputs['W3'], np.float32)
    as3 = np.asarray(inputs['a_src3'], np.float32); ad3 = np.asarray(inputs['a_dst3'], np.float32)
    b3 = np.asarray(inputs['b3'], np.float32)
    g3 = np.asarray(inputs['g3'], np.float32); be3 = np.asarray(inputs['be3'], np.float32)

    NPC = N // NC                      # nodes per core (exact: 50000/8=6250)
    NG = (NPC + 127) // 128            # groups per core (49)
    NPCP = NG * 128                    # padded nodes per core (6272)

    # ---- L1 tables (cnt trick) ----
    z1 = emb @ W1                                     # [32, 512]
    z1h = z1.reshape(TILE, H1, C1)
    al1t = np.einsum('thc,hc->th', z1h, as1)          # [32,8]
    ar1t = np.einsum('thc,hc->th', z1h, ad1)
    # E_tab[xd, h, t] = exp(lrelu(al1t[t,h] + ar1t[xd,h]))
    ee = al1t.T[None, :, :] + ar1t[:, :, None]        # [xd=32, h=8, t=32]
    ee = np.where(ee > 0, ee, 0.2 * ee)
    E_tab = np.exp(ee).astype(np.float32)             # [32, 8, 32]

    # cnt histogram over ALL edges incl self-loops
    src_all = np.concatenate([ei[0], np.arange(N)])
    dst_all = np.concatenate([ei[1], np.arange(N)])
    xs_all = x[src_all]
    cnt = np.zeros((N, TILE), np.float32)
    np.add.at(cnt, (dst_all, xs_all), 1.0)

    # ---- weight tables ----
    def wprime(W, a_s, a_d, H, C, pad_to):
        Fin = W.shape[0]
        As = np.zeros((H * C, H), np.float32)
        Ad = np.zeros((H * C, H), np.float32)
        for h in range(H):
            As[h * C:(h + 1) * C, h] = a_s[h]
            Ad[h * C:(h + 1) * C, h] = a_d[h]
        Wp = np.concatenate([W, W @ As, W @ Ad], axis=1)  # [Fin, H*C + 2H]
        out = np.zeros((Fin, pad_to), np.float32)
        out[:, :Wp.shape[1]] = Wp
        return out

    REC2 = 576   # 512 z + 8 al + 8 ar + 48 pad (f32)
    REC3 = 32    # 16 z + 1 al + 1 ar + 14 pad
    W2p = wprime(W2, as2, ad2, H2, C2, REC2)          # [512, 576]
    W3p = wprime(W3, as3, ad3, H3, C3, REC3)          # [512, 32]
    W2c = W2p.reshape(4, 128, REC2).copy()
    W3c = W3p.reshape(4, 128, REC3).copy()

    def bc(v, F):
        t = np.zeros((128, F), np.float32); t[:, :] = v[None, :F]; return t

    consts = dict(
        W2c=W2c, W3c=W3c,
        z1t=z1.astype(np.float32),                    # [32, 512]
        b1t=bc(b1, 512), g1t=bc(g1, 512), be1t=bc(be1, 512),
        b2t=bc(b2, 512), g2t=bc(g2, 512), be2t=bc(be2, 512),
        b3t=bc(b3, 16), g3t=bc(g3, 16), be3t=bc(be3, 16),
        iotaF=np.tile(np.arange(128, dtype=np.float32), (128, 1)),
        ident=np.eye(128, dtype=np.float32),
        onesc=np.ones((128, 1), np.float32),
    )

    # ---- per-core edge bucketing (non-self edges only) ----
    es, ed = ei[0], ei[1]
    core_of = ed // NPC
    grp_of = (ed % NPC) // 128
    # count per (core, group)
    counts = np.zeros((NC, NG), np.int64)
    np.add.at(counts, (core_of, grp_of), 1)
    Tg = np.maximum(1, ((counts.max(axis=0) + 127) // 128)).astype(np.int64)  # per-group tiles

    # gather index remap: node n -> row (n//NPC)*NPCP + n%NPC
    gidx_all = (es // NPC) * NPCP + (es % NPC)

    order = np.lexsort((es, grp_of, core_of))
    es_s, ed_s = es[order], ed[order]
    core_s, grp_s = core_of[order], grp_of[order]
    gidx_s = gidx_all[order]
    # boundaries per (core, group)
    starts = np.zeros((NC, NG), np.int64)
    flat = core_s * NG + grp_s
    bounds = np.searchsorted(flat, np.arange(NC * NG))
    starts = bounds.reshape(NC, NG)
    total = len(es_s)

    idx_src = np.zeros((NC, int(Tg.sum()) * 128), np.int32)
    dstloc = np.full((NC, int(Tg.sum()) * 128), 200.0, np.float32)
    toff = np.concatenate([[0], np.cumsum(Tg)]).astype(np.int64)  # tile offsets per group
    for c in range(NC):
        for g in range(NG):
            s = starts[c, g]
            e = starts[c, g + 1] if g + 1 < NG else (starts[c + 1, 0] if c + 1 < NC else total)
            n = e - s
            o = int(toff[g]) * 128
            cap = int(Tg[g]) * 128
            assert n <= cap, (c, g, n, cap)
            idx_src[c, o:o + n] = gidx_s[s:e]
            dstloc[c, o:o + n] = (ed_s[s:e] % NPC) % 128
    # reshape per group tile-major: slot j within group -> (tile j//128? ) We store
    # edge slot j at [tile=j//128 ... wait gather layout: out[p, t] = row idx[t*128+p]
    # => idx array per group laid out [T,128] with tile-major flattening, and the
    # SBUF idx tile loaded as [128, T] must be the transpose.
    NTT = int(Tg.sum())
    idx_src = idx_src.reshape(NC, NTT, 128)
    dstloc = dstloc.reshape(NC, NTT, 128)
    # SBUF-friendly layout [128, NTT]
    idx_srcT = np.ascontiguousarray(idx_src.transpose(0, 2, 1))   # [NC, 128, NTT]
    dstlocT = np.ascontiguousarray(dstloc.transpose(0, 2, 1))     # [NC, 128, NTT]

    # ---- per-core node arrays ----
    percore = []
    for c in range(NC):
        lo, hi = c * NPC, (c + 1) * NPC
        cntc = np.zeros((NPCP, TILE), np.float32)
        cntc[:NPC] = cnt[lo:hi]
        cntc[NPC:, 0] = 1.0  # pad rows: avoid 0/0
        Ec = np.zeros((NPCP, H1 * TILE), np.float32)
        Ec[:NPC] = E_tab[x[lo:hi]].reshape(NPC, H1 * TILE)
        Ec[NPC:] = 1.0
        batchc = np.full((NPCP, 1), 200.0, np.float32)
        batchc[:NPC, 0] = batch[lo:hi]
        percore.append(dict(
            cntc=cntc, Ec=Ec, batchc=batchc,
            idxs=idx_srcT[c], dls=dstlocT[c],
        ))

    meta = dict(N=N, E=E, G=G, NC=NC, NPC=NPC, NG=NG, NPCP=NPCP, Tg=Tg.tolist(),
                toff=toff.tolist(), REC2=REC2, REC3=REC3, H1=H1, C1=C1, H2=H2,
                C2=C2, H3=H3, C3=C3, TILE=TILE)
    host = dict(fcW1=np.asarray(inputs['fcW1'], np.float32),
                fcb1=np.asarray(inputs['fcb1'], np.float32),
                fcW2=np.asarray(inputs['fcW2'], np.float32),
                fcb2=np.asarray(inputs['fcb2'], np.float32),
                batch=batch)
    return consts, percore, meta, host


def layer_norm_elu(nc, pool, y, g_t, be_t, F, epsc=None):
    """In SBUF: y [128,F] -> elu(LN(y)*g+be). In-place heavy; returns new tile."""
    s1 = pool.tile([128, 1], F32, tag="ln_s1")
    nc.vector.tensor_reduce(out=s1[:], in_=y[:], axis=AX.X, op=ALU.add)
    m2 = pool.tile([128, 1], F32, tag="ln_m2")
    nc.vector.tensor_scalar_mul(out=m2[:], in0=s1[:], scalar1=-1.0 / F)
    sq = pool.tile([128, F], F32, tag="ln_sq")
    ss = pool.tile([128, 1], F32, tag="ln_ss")
    nc.scalar.activation(out=sq[:], in_=y[:], func=ACT.Square, bias=m2[:, :1],
                         accum_out=ss[:])
    sd = pool.tile([128, 1], F32, tag="ln_sd")
    nc.scalar.activation(out=sd[:], in_=ss[:], func=ACT.Sqrt, bias=epsc[:, :1], scale=1.0 / F)
    rs = pool.tile([128, 1], F32, tag="ln_rs")
    nc.vector.reciprocal(out=rs[:], in_=sd[:])
    # y <- (y - m) * istd ; then *g ; then +be   (in place)
    nc.vector.tensor_scalar(out=y[:], in0=y[:], scalar1=m2[:, :1], scalar2=rs[:, :1],
                            op0=ALU.add, op1=ALU.mult)
    nc.vector.tensor_tensor(out=y[:], in0=y[:], in1=g_t[:, :F], op=ALU.mult)
    nc.vector.tensor_tensor(out=y[:], in0=y[:], in1=be_t[:, :F], op=ALU.add)
    # ELU = max(x,0) + exp(min(x,0)) - 1 ; sq reused as scratch
    nc.vector.tensor_scalar_min(out=sq[:], in0=y[:], scalar1=0.0)
    nc.scalar.activation(out=sq[:], in_=sq[:], func=ACT.Exp)
    h = pool.tile([128, F], F32, tag="elu_h")
    nc.vector.tensor_scalar(out=h[:], in0=y[:], scalar1=0.0, scalar2=-1.0,
                            op0=ALU.max, op1=ALU.add)
    nc.vector.tensor_tensor(out=h[:], in0=h[:], in1=sq[:], op=ALU.add)
    return h


def transpose_128(nc, sb, pst, src_ap, ident, tag):
    """PE-transpose a [128,128] SBUF slice -> new SBUF tile."""
    pt = pst.tile([128, 128], F32, tag="tp_ps", space="PSUM")
    nc.tensor.transpose(out=pt[:], in_=src_ap, identity=ident[:])
    st = sb.tile([128, 128], F32, tag="tp_sb")
    nc.vector.tensor_copy(out=st[:], in_=pt[:])
    return st


def build(meta):
    NC, NG, NPCP = meta['NC'], meta['NG'], meta['NPCP']
    Tg, toff = meta['Tg'], meta['toff']
    NTT = toff[-1]
    REC2, REC3 = meta['REC2'], meta['REC3']
    G = meta['G']
    TILE, H1 = meta['TILE'], meta['H1']
    NFULL = NC * NPCP

    nc = bacc.Bacc("TRN2", num_devices=NC)
    # inputs
    t_cnt = nc.dram_tensor("cntc", [NPCP, TILE], F32, kind="ExternalInput")
    t_E = nc.dram_tensor("Ec", [NPCP, H1 * TILE], F32, kind="ExternalInput")
    t_bat = nc.dram_tensor("batchc", [NPCP, 1], F32, kind="ExternalInput")
    t_idx = nc.dram_tensor("idxs", [128, NTT], I32, kind="ExternalInput")
    t_dl = nc.dram_tensor("dls", [128, NTT], F32, kind="ExternalInput")
    t_W2c = nc.dram_tensor("W2c", [4, 128, REC2], F32, kind="ExternalInput")
    t_W3c = nc.dram_tensor("W3c", [4, 128, REC3], F32, kind="ExternalInput")
    t_z1t = nc.dram_tensor("z1t", [TILE, 512], F32, kind="ExternalInput")
    cn = {}
    for nm, sh in [("b1t", 512), ("g1t", 512), ("be1t", 512), ("b2t", 512),
                   ("g2t", 512), ("be2t", 512), ("b3t", 16), ("g3t", 16), ("be3t", 16)]:
        cn[nm] = nc.dram_tensor(nm, [128, sh], F32, kind="ExternalInput")
    t_iota = nc.dram_tensor("iotaF", [128, 128], F32, kind="ExternalInput")
    t_id = nc.dram_tensor("ident", [128, 128], F32, kind="ExternalInput")
    t_ones = nc.dram_tensor("onesc", [128, 1], F32, kind="ExternalInput")
    t_out = nc.dram_tensor("part", [G, 17], F32, kind="ExternalOutput")

    with tile.TileContext(nc) as tc:
        with tc.tile_pool(name="const", bufs=1) as cp, \
             tc.tile_pool(name="sb", bufs=2) as sb, \
             tc.tile_pool(name="gbuf", bufs=2) as gb, \
             tc.tile_pool(name="ps", bufs=1, space="PSUM") as ps, \
             tc.tile_pool(name="pst", bufs=2, space="PSUM") as pst, \
             tc.tile_pool(name="pacc", bufs=1, space="PSUM") as pacc, \
             tc.tile_pool(name="dram", bufs=1, space="DRAM") as dp:

            # ---- const loads ----
            C = {}
            for nm, src, shp in [("iotaF", t_iota, [128, 128]), ("ident", t_id, [128, 128]),
                                 ("z1t", t_z1t, [TILE, 512]), ("onesc", t_ones, [128, 1])]:
                C[nm] = cp.tile(shp, F32, tag="c_" + nm, name="c_" + nm)
                nc.sync.dma_start(out=C[nm][:], in_=src[:])
            for nm in cn:
                F = 512 if nm[-2] != '3' else 16
                C[nm] = cp.tile([128, F], F32, tag="c_" + nm, name="c_" + nm)
                nc.sync.dma_start(out=C[nm][:], in_=cn[nm][:])
            W2s = cp.tile([128, 4 * REC2], F32)
            nc.sync.dma_start(out=W2s[:].rearrange("p (a b) -> p a b", a=4), in_=t_W2c[:].rearrange("a p b -> p a b"))
            epsc = cp.tile([128, 1], F32, name="epsc")
            nc.vector.memset(epsc[:], EPS)
            W3s = cp.tile([128, 4 * REC3], F32)
            nc.sync.dma_start(out=W3s[:].rearrange("p (a b) -> p a b", a=4), in_=t_W3c[:].rearrange("a p b -> p a b"))

            rec2_sh = dp.tile([NPCP, REC2], F32)
            rec2_full = dp.tile([NFULL, REC2], F32, addr_space="Shared")
            rec3_sh = dp.tile([NPCP, REC3], F32)
            rec3_full = dp.tile([NFULL, REC3], F32, addr_space="Shared")

            # ================= L1 + phaseA(L2) =================
            for g in range(NG):
                r0 = g * 128
                cg = sb.tile([128, TILE], F32, tag="cg")
                nc.sync.dma_start(out=cg[:], in_=t_cnt[r0:r0 + 128, :])
                Eg = sb.tile([128, H1, TILE], F32, tag="Eg")
                nc.sync.dma_start(out=Eg[:, :, :], in_=t_E[r0:r0 + 128, :].rearrange("p (h t) -> p h t", h=H1))
                M = sb.tile([128, H1, TILE], F32, tag="M")
                nc.vector.tensor_tensor(out=M[:, :, :], in0=Eg[:, :, :],
                                        in1=cg[:, None, :].to_broadcast([128, H1, TILE]),
                                        op=ALU.mult)
                s = sb.tile([128, H1], F32, tag="s")
                nc.vector.tensor_reduce(out=s[:], in_=M[:, :, :], axis=AX.X, op=ALU.add)
                rs = sb.tile([128, H1], F32, tag="rs")
                nc.vector.reciprocal(out=rs[:], in_=s[:])
                nc.vector.tensor_tensor(out=M[:, :, :], in0=M[:, :, :],
                                        in1=rs[:, :, None].to_broadcast([128, H1, TILE]),
                                        op=ALU.mult)
                P = M
                pO = ps.tile([128, 512], F32, tag="pacc_main", space="PSUM")
                for h in range(H1):
                    ptp = pst.tile([128, 128], F32, tag="tp_ps", space="PSUM")
                    nc.tensor.transpose(out=ptp[:TILE, :], in_=P[:, h, :], identity=C["ident"][:])
                    PT = sb.tile([TILE, 128], F32, tag="PT")
                    nc.vector.tensor_copy(out=PT[:], in_=ptp[:TILE, :])
                    nc.tensor.matmul(out=pO[:, h * 64:(h + 1) * 64], lhsT=PT[:],
                                     rhs=C["z1t"][:, h * 64:(h + 1) * 64],
                                     start=True, stop=True)
                y = sb.tile([128, 512], F32, tag="y1")
                nc.vector.tensor_tensor(out=y[:], in0=pO[:], in1=C["b1t"][:], op=ALU.add)
                h1 = layer_norm_elu(nc, sb, y, C["g1t"], C["be1t"], 512, epsc)
                # transpose h1 -> 4 chunks, phase-A W2'
                z2p = ps.tile([128, 512], F32, tag="pz", space="PSUM")
                z2pb = ps.tile([128, 64], F32, tag="z2pb", space="PSUM")
                for k in range(4):
                    hT = transpose_128(nc, sb, pst, h1[:, k * 128:(k + 1) * 128], C["ident"], "h1T")
                    nc.tensor.matmul(out=z2p[:], lhsT=hT[:], rhs=W2s[:, k * REC2:k * REC2 + 512],
                                     start=(k == 0), stop=(k == 3))
                    nc.tensor.matmul(out=z2pb[:], lhsT=hT[:], rhs=W2s[:, k * REC2 + 512:(k + 1) * REC2],
                                     start=(k == 0), stop=(k == 3))
                zs = sb.tile([128, REC2], F32, tag="zs")
                nc.vector.tensor_copy(out=zs[:, :512], in_=z2p[:])
                nc.vector.tensor_copy(out=zs[:, 512:], in_=z2pb[:])
                nc.sync.dma_start(out=rec2_sh[r0:r0 + 128, :], in_=zs[:])

            nc.gpsimd.collective_compute(
                "AllGather", ALU.bypass, replica_groups=[list(range(NC))],
                ins=[rec2_sh.opt()], outs=[rec2_full.opt()])

            # ================= L2 + phaseA(L3) =================
            for g in range(NG):
                r0 = g * 128
                T = Tg[g]
                o0 = toff[g]
                ig = sb.tile([128, T], I32, tag="ig")
                nc.sync.dma_start(out=ig[:], in_=t_idx[:, o0:o0 + T])
                dl = sb.tile([128, T], F32, tag="dl")
                nc.sync.dma_start(out=dl[:], in_=t_dl[:, o0:o0 + T])
                zg = sb.tile([128, REC2], F32, tag="zg")
                nc.sync.dma_start(out=zg[:], in_=rec2_sh[r0:r0 + 128, :])
                Gt = gb.tile([128, T, REC2], F32, tag="G")
                for t in range(T):
                    nc.gpsimd.indirect_dma_start(
                        out=Gt[:, t, :], out_offset=None, in_=rec2_full[:],
                        in_offset=bass.IndirectOffsetOnAxis(ap=ig[:, t:t + 1], axis=0))
                S = gb.tile([128, T, 128], F32, tag="S")
                nc.vector.tensor_tensor(
                    out=S[:, :, :],
                    in0=C["iotaF"][:, None, :].to_broadcast([128, T, 128]),
                    in1=dl[:, :, None].to_broadcast([128, T, 128]),
                    op=ALU.is_equal)
                H2x, C2x = 4, 128
                pAR = ps.tile([128, T * H2x], F32, tag="pAR", space="PSUM")
                for t in range(T):
                    STt = transpose_128(nc, sb, pst, S[:, t, :], C["ident"], "ST")
                    nc.tensor.matmul(out=pAR[:, t * H2x:(t + 1) * H2x], lhsT=STt[:],
                                     rhs=zg[:, 516:520], start=True, stop=True)
                eL = sb.tile([128, T * H2x], F32, tag="eL")
                nc.vector.tensor_tensor(
                    out=eL[:].rearrange("p (t h) -> p t h", h=H2x),
                    in0=Gt[:, :, 512:516], in1=pAR[:].rearrange("p (t h) -> p t h", h=H2x),
                    op=ALU.add)
                eA = sb.tile([128, T * H2x], F32, tag="eA")
                nc.vector.tensor_scalar_mul(out=eA[:], in0=eL[:], scalar1=0.2)
                nc.vector.tensor_tensor(out=eA[:], in0=eL[:], in1=eA[:], op=ALU.max)
                EX = sb.tile([128, T * H2x], F32, tag="EX")
                nc.scalar.activation(out=EX[:], in_=eA[:], func=ACT.Exp)
                # scale z-part of G by EX (per head block of C2x)
                nc.vector.tensor_tensor(
                    out=Gt[:, :, :512].rearrange("p t (h c) -> p t h c", h=H2x),
                    in0=Gt[:, :, :512].rearrange("p t (h c) -> p t h c", h=H2x),
                    in1=EX[:].rearrange("p (t h) -> p t h", h=H2x)[:, :, :, None]
                        .to_broadcast([128, T, H2x, C2x]),
                    op=ALU.mult)
                pMain = ps.tile([128, 512], F32, tag="pacc_main", space="PSUM")
                pS = ps.tile([128, H2x], F32, tag="pacc_s", space="PSUM")
                for t in range(T):
                    nc.tensor.matmul(out=pMain[:], lhsT=S[:, t, :], rhs=Gt[:, t, :512],
                                     start=(t == 0), stop=(t == T - 1))
                    nc.tensor.matmul(out=pS[:], lhsT=S[:, t, :], rhs=EX[:, t * H2x:(t + 1) * H2x],
                                     start=(t == 0), stop=(t == T - 1))
                # self-loop
                eSl = sb.tile([128, H2x], F32, tag="eSl")
                nc.vector.tensor_tensor(out=eSl[:], in0=zg[:, 512:516], in1=zg[:, 516:520], op=ALU.add)
                eSa = sb.tile([128, H2x], F32, tag="eSa")
                nc.vector.tensor_scalar_mul(out=eSa[:], in0=eSl[:], scalar1=0.2)
                nc.vector.tensor_tensor(out=eSa[:], in0=eSl[:], in1=eSa[:], op=ALU.max)
                exS = sb.tile([128, H2x], F32, tag="exS")
                nc.scalar.activation(out=exS[:], in_=eSa[:], func=ACT.Exp)
                selfc = sb.tile([128, 512], F32, tag="selfc")
                nc.vector.tensor_tensor(
                    out=selfc[:].rearrange("p (h c) -> p h c", h=H2x),
                    in0=zg[:, :512].rearrange("p (h c) -> p h c", h=H2x),
                    in1=exS[:, :, None].to_broadcast([128, H2x, C2x]), op=ALU.mult)
                nc.vector.tensor_tensor(out=selfc[:], in0=pMain[:], in1=selfc[:], op=ALU.add)
                sS = sb.tile([128, H2x], F32, tag="sS")
                nc.vector.tensor_tensor(out=sS[:], in0=pS[:], in1=exS[:], op=ALU.add)
                rS = sb.tile([128, H2x], F32, tag="rS")
                nc.vector.reciprocal(out=rS[:], in_=sS[:])
                nc.vector.tensor_tensor(
                    out=selfc[:].rearrange("p (h c) -> p h c", h=H2x),
                    in0=selfc[:].rearrange("p (h c) -> p h c", h=H2x),
                    in1=rS[:, :, None].to_broadcast([128, H2x, C2x]), op=ALU.mult)
                nc.vector.tensor_tensor(out=selfc[:], in0=selfc[:], in1=C["b2t"][:], op=ALU.add)
                h2 = layer_norm_elu(nc, sb, selfc, C["g2t"], C["be2t"], 512, epsc)
                z3p = ps.tile([128, REC3], F32, tag="pz", space="PSUM")
                for k in range(4):
                    hT = transpose_128(nc, sb, pst, h2[:, k * 128:(k + 1) * 128], C["ident"], "h2T")
                    nc.tensor.matmul(out=z3p[:], lhsT=hT[:], rhs=W3s[:, k * REC3:(k + 1) * REC3],
                                     start=(k == 0), stop=(k == 3))
                z3s = sb.tile([128, REC3], F32, tag="z3s")
                nc.vector.tensor_copy(out=z3s[:], in_=z3p[:])
                nc.sync.dma_start(out=rec3_sh[r0:r0 + 128, :], in_=z3s[:])

            nc.gpsimd.collective_compute(
                "AllGather", ALU.bypass, replica_groups=[list(range(NC))],
                ins=[rec3_sh.opt()], outs=[rec3_full.opt()])

            # ================= L3 + pooling =================
            pPool = pacc.tile([128, 17], F32, tag="pPool", space="PSUM")
            for g in range(NG):
                r0 = g * 128
                T = Tg[g]
                o0 = toff[g]
                ig = sb.tile([128, T], I32, tag="ig3")
                nc.sync.dma_start(out=ig[:], in_=t_idx[:, o0:o0 + T])
                dl = sb.tile([128, T], F32, tag="dl3")
                nc.sync.dma_start(out=dl[:], in_=t_dl[:, o0:o0 + T])
                zg = sb.tile([128, REC3], F32, tag="zg3")
                nc.sync.dma_start(out=zg[:], in_=rec3_sh[r0:r0 + 128, :])
                bg = sb.tile([128, 1], F32, tag="bg")
                nc.sync.dma_start(out=bg[:], in_=t_bat[r0:r0 + 128, :])
                Gt = gb.tile([128, T, REC3], F32, tag="G")
                for t in range(T):
                    nc.gpsimd.indirect_dma_start(
                        out=Gt[:, t, :], out_offset=None, in_=rec3_full[:],
                        in_offset=bass.IndirectOffsetOnAxis(ap=ig[:, t:t + 1], axis=0))
                S = gb.tile([128, T, 128], F32, tag="S")
                nc.vector.tensor_tensor(
                    out=S[:, :, :],
                    in0=C["iotaF"][:, None, :].to_broadcast([128, T, 128]),
                    in1=dl[:, :, None].to_broadcast([128, T, 128]),
                    op=ALU.is_equal)
                pAR = ps.tile([128, T], F32, tag="pAR", space="PSUM")
                for t in range(T):
                    STt = transpose_128(nc, sb, pst, S[:, t, :], C["ident"], "ST3")
                    nc.tensor.matmul(out=pAR[:, t:t + 1], lhsT=STt[:],
                                     rhs=zg[:, 17:18], start=True, stop=True)
                eL = sb.tile([128, T], F32, tag="eL3")
                nc.vector.tensor_tensor(out=eL[:], in0=Gt[:, :, 16], in1=pAR[:], op=ALU.add)
                eA = sb.tile([128, T], F32, tag="eA3")
                nc.vector.tensor_scalar_mul(out=eA[:], in0=eL[:], scalar1=0.2)
                nc.vector.tensor_tensor(out=eA[:], in0=eL[:], in1=eA[:], op=ALU.max)
                EX = sb.tile([128, T], F32, tag="EX3")
                nc.scalar.activation(out=EX[:], in_=eA[:], func=ACT.Exp)
                nc.vector.tensor_tensor(
                    out=Gt[:, :, :16], in0=Gt[:, :, :16],
                    in1=EX[:, :, None].to_broadcast([128, T, 16]), op=ALU.mult)
                pM3 = ps.tile([128, 16], F32, tag="pacc_main", space="PSUM")
                pS3 = ps.tile([128, 1], F32, tag="pacc_s", space="PSUM")
                for t in range(T):
                    nc.tensor.matmul(out=pM3[:], lhsT=S[:, t, :], rhs=Gt[:, t, :16],
                                     start=(t == 0), stop=(t == T - 1))
                    nc.tensor.matmul(out=pS3[:], lhsT=S[:, t, :], rhs=EX[:, t:t + 1],
                                     start=(t == 0), stop=(t == T - 1))
                eSl = sb.tile([128, 1], F32, tag="eSl3")
                nc.vector.tensor_tensor(out=eSl[:], in0=zg[:, 16:17], in1=zg[:, 17:18], op=ALU.add)
                eSa = sb.tile([128, 1], F32, tag="eSa3")
                nc.vector.tensor_scalar_mul(out=eSa[:], in0=eSl[:], scalar1=0.2)
                nc.vector.tensor_tensor(out=eSa[:], in0=eSl[:], in1=eSa[:], op=ALU.max)
                exS = sb.tile([128, 1], F32, tag="exS3")
                nc.scalar.activation(out=exS[:], in_=eSa[:], func=ACT.Exp)
                selfc = sb.tile([128, 16], F32, tag="selfc3")
                nc.vector.tensor_scalar(out=selfc[:], in0=zg[:, :16], scalar1=exS[:, :1],
                                        scalar2=None, op0=ALU.mult)
                nc.vector.tensor_tensor(out=selfc[:], in0=pM3[:], in1=selfc[:], op=ALU.add)
                sS = sb.tile([128, 1], F32, tag="sS3")
                nc.vector.tensor_tensor(out=sS[:], in0=pS3[:], in1=exS[:], op=ALU.add)
                rS = sb.tile([128, 1], F32, tag="rS3")
                nc.vector.reciprocal(out=rS[:], in_=sS[:])
                nc.vector.tensor_scalar(out=selfc[:], in0=selfc[:], scalar1=rS[:, :1],
                                        scalar2=None, op0=ALU.mult)
                nc.vector.tensor_tensor(out=selfc[:], in0=selfc[:], in1=C["b3t"][:], op=ALU.add)
                h3 = layer_norm_elu(nc, sb, selfc, C["g3t"], C["be3t"], 16, epsc)
                OB = sb.tile([128, G], F32, tag="OB")
                nc.vector.tensor_tensor(
                    out=OB[:], in0=C["iotaF"][:, :G],
                    in1=bg[:, :1].to_broadcast([128, G]), op=ALU.is_equal)
                h3w = sb.tile([128, 17], F32, tag="h3w")
                nc.vector.tensor_copy(out=h3w[:, :16], in_=h3[:])
                nc.vector.memset(h3w[:, 16:17], 1.0)
                nc.tensor.matmul(out=pPool[:G, :17], lhsT=OB[:], rhs=h3w[:],
                                 start=(g == 0), stop=(g == NG - 1))
            po = sb.tile([128, 17], F32, tag="po")
            nc.vector.tensor_copy(out=po[:G, :], in_=pPool[:G, :])
            nc.sync.dma_start(out=t_out[:, :], in_=po[:G, :])
    nc.finalize()
    return nc


def run(inputs, N, E, G, NC, runner, TILE=32, EMB=16):
    consts, percore, meta, host = host_prep(inputs, N, E, G, NC, TILE, EMB)
    nc = build(meta)
    in_maps = []
    for c in range(NC):
        m = dict(consts)
        m.update(percore[c])
        in_maps.append(m)
    results = runner(nc, in_maps)
    parts = np.stack([r["part"] for r in results])  # [NC, G, 17]
    tot = parts.sum(axis=0)
    pooled = tot[:, :16] / np.maximum(tot[:, 16:17], 1.0)
    h = np.maximum(pooled @ host['fcW1'] + host['fcb1'], 0.0)
    return (h @ host['fcW2'] + host['fcb2']).astype(np.float32)


# ======================= kernel entry =======================
N_FULL, E_FULL, G_FULL, NC_FULL = 50000, 800000, 50, 8
_CACHE = {}


def _hw_runner(nc, in_maps):
    from concourse.bass_utils import run_bass_kernel_spmd
    res = run_bass_kernel_spmd(nc, in_maps, core_ids=list(range(len(in_maps))))
    return res.results


def kernel(**inputs):
    consts, percore, meta, host = host_prep(inputs, N_FULL, E_FULL, G_FULL, NC_FULL)
    key = tuple(meta['Tg'])
    if key not in _CACHE:
        _CACHE[key] = build(meta)
    nc = _CACHE[key]
    in_maps = []
    for c in range(NC_FULL):
        m = dict(consts)
        m.update(percore[c])
        in_maps.append(m)
    results = _hw_runner(nc, in_maps)
    parts = np.stack([r["part"] for r in results])
    tot = parts.sum(axis=0)
    pooled = tot[:, :16] / np.maximum(tot[:, 16:17], 1.0)
    h = np.maximum(pooled @ host['fcW1'] + host['fcb1'], 0.0)
    return (h @ host['fcW2'] + host['fcb2']).astype(np.float32)



# revision 10
# speedup vs baseline: 1.1867x; 1.1867x over previous
"""MinamoTopoModel GAT kernel v2: bf16 + dma_gather + host-precomputed
selection matrices + chunked AllGathers.

Per 8-core SPMD, dst-sharded (6250 nodes/core, 49 groups of 128):
  L1: cnt-histogram trick, transposed (M^T on (head,tile) partitions) ->
      matmul-reductions, no per-head PE transposes.
  L2/L3: per-group fused dma_gather of src records (bf16, 640B/256B rows),
      host-precomputed S/S^T selection matrices (u8 -> bf16 cast DMA),
      channel-interleaved records so the softmax scale is one 2x DVE op,
      scatter + denominator matmuls share the S stationary.
  Records published via 4-chunk AllGathers overlapped with compute.
  Pooling -> per-core [50,17] partials; final FC on host.
"""
import numpy as np
import ml_dtypes
import concourse.bacc as bacc
import concourse.bass as bass
import concourse.mybir as mybir
import concourse.tile as tile

F32 = mybir.dt.float32
BF16 = mybir.dt.bfloat16
I16 = mybir.dt.int16
U8 = mybir.dt.uint8
AX = mybir.AxisListType
ALU = mybir.AluOpType
ACT = mybir.ActivationFunctionType
EPS = 1e-5
BF = ml_dtypes.bfloat16

N, E, G, NC = 50000, 800000, 50, 8
NPC, NG, NPCP = 6250, 49, 6272
NFULL = NC * NPCP
RB = np.array([0, 1568, 3136, 4704, 6272])
CSZ = 12544  # = 8*1568, rows per chunk table
REC2, REC3 = 640, 128
H1, C1, H2, C2, H3, C3 = 8, 64, 4, 128, 1, 16
TILE = 32


def host_prep(inputs):
    x = np.asarray(inputs['x']).astype(np.int64)
    ei = np.asarray(inputs['edge_index']).astype(np.int64)
    batch = np.asarray(inputs['batch']).astype(np.int64)
    emb = np.asarray(inputs['emb'], np.float32)
    W1 = np.asarray(inputs['W1'], np.float32)
    as1 = np.asarray(inputs['a_src1'], np.float32)
    ad1 = np.asarray(inputs['a_dst1'], np.float32)
    b1 = np.asarray(inputs['b1'], np.float32)
    g1 = np.asarray(inputs['g1'], np.float32)
    be1 = np.asarray(inputs['be1'], np.float32)
    W2 = np.asarray(inputs['W2'], np.float32)
    as2 = np.asarray(inputs['a_src2'], np.float32)
    ad2 = np.asarray(inputs['a_dst2'], np.float32)
    b2 = np.asarray(inputs['b2'], np.float32)
    g2 = np.asarray(inputs['g2'], np.float32)
    be2 = np.asarray(inputs['be2'], np.float32)
    W3 = np.asarray(inputs['W3'], np.float32)
    as3 = np.asarray(inputs['a_src3'], np.float32)
    ad3 = np.asarray(inputs['a_dst3'], np.float32)
    b3 = np.asarray(inputs['b3'], np.float32)
    g3 = np.asarray(inputs['g3'], np.float32)
    be3 = np.asarray(inputs['be3'], np.float32)

    ar512 = np.arange(512)
    perm1 = (ar512 % H1) * C1 + ar512 // H1   # interleaved col n <- orig col
    perm2 = (ar512 % H2) * C2 + ar512 // H2

    # ---- L1 tables ----
    z1 = emb @ W1                                   # [32, 512]
    z1h = z1.reshape(TILE, H1, C1)
    al1t = np.einsum('thc,hc->th', z1h, as1)
    ar1t = np.einsum('thc,hc->th', z1h, ad1)
    ee = al1t.T[None, :, :] + ar1t[:, :, None]      # [xd, h, t]
    ee = np.where(ee > 0, ee, 0.2 * ee)
    E_tab = np.exp(ee).astype(np.float32)           # [32, 8, 32]

    z1i = z1[:, perm1]
    rows = np.arange(128)
    hh, tt = rows // 32, rows % 32
    colh = ar512 % H1
    z1w_lo = np.where(colh[None, :] == hh[:, None], z1i[tt, :], 0.0).astype(np.float32)
    z1w_hi = np.where(colh[None, :] == (hh + 4)[:, None], z1i[tt, :], 0.0).astype(np.float32)
    ind_lo = (hh[:, None] == np.arange(8)[None, :]).astype(np.float32)
    ind_hi = ((hh + 4)[:, None] == np.arange(8)[None, :]).astype(np.float32)

    src_all = np.concatenate([ei[0], np.arange(N)])
    dst_all = np.concatenate([ei[1], np.arange(N)])
    cnt = np.zeros((N, TILE), np.float32)
    np.add.at(cnt, (dst_all, x[src_all]), 1.0)

    # ---- W2/W3 with interleave ----
    W2z = W2[:, perm2]
    W2r3 = W2.reshape(512, H2, C2)
    W2a = np.einsum('khc,hc->kh', W2r3, as2)
    W2r = np.einsum('khc,hc->kh', W2r3, ad2)
    Wf2 = np.concatenate([W2z, W2a, W2r], 1)[perm1, :]          # [512, 520]
    W2s = np.concatenate([Wf2[k * 128:(k + 1) * 128] for k in range(4)], 1)

    W3a = (W3.reshape(512, 16) @ as3[0])[:, None]
    W3r = (W3.reshape(512, 16) @ ad3[0])[:, None]
    Wf3 = np.concatenate([W3, W3a, W3r, np.zeros((512, 2), np.float32)], 1)[perm2, :]
    W3s = np.concatenate([Wf3[k * 128:(k + 1) * 128] for k in range(4)], 1)  # [128, 80]

    def bcast(v):
        return np.tile(v[None, :], (128, 1)).astype(np.float32)

    consts = dict(
        z1w_lo=z1w_lo, z1w_hi=z1w_hi, ind_lo=ind_lo, ind_hi=ind_hi,
        W2s=W2s, W3s=W3s,
        b1t=bcast(b1[perm1]), g1t=bcast(g1[perm1]), be1t=bcast(be1[perm1]),
        b2t=bcast(b2[perm2]), g2t=bcast(g2[perm2]), be2t=bcast(be2[perm2]),
        b3t=bcast(b3), g3t=bcast(g3), be3t=bcast(be3),
        iotaF50=np.tile(np.arange(64, dtype=np.float32), (128, 1)),
    )

    # ---- edges ----
    sz = np.diff(RB)
    off = 8 * RB[:-1]

    def grow(c, r):
        k = np.searchsorted(RB, r, side='right') - 1
        return off[k] + c * sz[k] + (r - RB[k])

    es, ed = ei[0], ei[1]
    cs, rsr = es // NPC, es % NPC
    ck = np.searchsorted(RB, rsr, side='right') - 1          # src chunk 0..3
    cidx = cs * 1568 + (rsr - RB[ck])                         # row in chunk table
    cd, rd = ed // NPC, ed % NPC
    gd, dl = rd // 128, rd % 128
    ncnt = np.zeros((NC, NG, 4), np.int64)
    np.add.at(ncnt, (cd, gd, ck), 1)
    Tc = np.ceil(ncnt.max(0) / 128).astype(int)               # [NG, 4]
    Tg = Tc.sum(1)
    toff = np.concatenate([[0], np.cumsum(Tg)]).astype(int)
    NTT = int(toff[-1])

    order = np.lexsort((cidx, ck, gd, cd))
    cidx_s, dl_s = cidx[order], dl[order]
    cd_s, gd_s, ck_s = cd[order], gd[order], ck[order]
    key = (cd_s * NG + gd_s) * 4 + ck_s
    bounds = np.searchsorted(key, np.arange(NC * NG * 4 + 1))

    percore = []
    for c in range(NC):
        idx16 = np.zeros((16, NTT * 8), np.int16)
        Su8 = np.zeros((128, NTT * 128), np.uint8)
        STu8 = np.zeros((128, NTT * 128), np.uint8)
        for g in range(NG):
            base4 = (c * NG + g) * 4
            tbase = int(toff[g])
            for k in range(4):
                Tk = int(Tc[g, k])
                s, e = bounds[base4 + k], bounds[base4 + k + 1]
                n = e - s
                if Tk == 0:
                    assert n == 0
                    continue
                L = np.zeros(Tk * 128, np.int64)
                L[:n] = cidx_s[s:e]
                idx16[:, tbase * 8:(tbase + Tk) * 8] = \
                    L.reshape(-1, 16).T.astype(np.int16)
                p_ = np.arange(n) % 128
                t_ = tbase + np.arange(n) // 128
                d_ = dl_s[s:e]
                Su8[p_, t_ * 128 + d_] = 1
                STu8[d_, t_ * 128 + p_] = 1
                tbase += Tk
        lo_n, hi_n = c * NPC, (c + 1) * NPC
        cntc = np.zeros((NPCP, TILE), np.float32)
        cntc[:NPC] = cnt[lo_n:hi_n]
        cntc[NPC:, 0] = 1.0
        cntT4 = np.tile(np.ascontiguousarray(cntc.T), (4, 1))    # [128, NPCP]
        ETf = np.ones((8, 32, NPCP), np.float32)
        ETf[:, :, :NPC] = np.moveaxis(E_tab[x[lo_n:hi_n]], 0, -1)
        batchc = np.full((NPCP,), 200.0, np.float32)
        batchc[:NPC] = batch[lo_n:hi_n]
        percore.append(dict(
            cntT4=cntT4.astype(BF),
            ETlo=ETf[0:4].reshape(128, NPCP).astype(BF),
            EThi=ETf[4:8].reshape(128, NPCP).astype(BF),
            idx16=np.tile(idx16, (8, 1)),
            Su8=Su8, STu8=STu8,
            batchc=np.ascontiguousarray(batchc.reshape(NG, 128).T),
        ))

    for k in consts:
        if k != 'iotaF50':
            consts[k] = consts[k].astype(BF)

    meta = dict(Tc=Tc.tolist(), NTT=NTT)
    host = dict(fcW1=np.asarray(inputs['fcW1'], np.float32),
                fcb1=np.asarray(inputs['fcb1'], np.float32),
                fcW2=np.asarray(inputs['fcW2'], np.float32),
                fcb2=np.asarray(inputs['fcb2'], np.float32))
    return consts, percore, meta, host


def ln_elu(nc, sb, y, g_t, be_t, Fd, epsc, tag):
    """y [128,Fd] bf16 in SBUF -> elu(LN(y)*g+be) as new bf16 tile."""
    s1 = sb.tile([128, 1], F32, tag="ln_s1")
    nc.vector.tensor_reduce(out=s1[:], in_=y[:], axis=AX.X, op=ALU.add)
    m2 = sb.tile([128, 1], F32, tag="ln_m2")
    nc.vector.tensor_scalar_mul(out=m2[:], in0=s1[:], scalar1=-1.0 / Fd)
    sq = sb.tile([128, Fd], BF16, tag="ln_sq")
    ss = sb.tile([128, 1], F32, tag="ln_ss")
    nc.scalar.activation(out=sq[:], in_=y[:], func=ACT.Square, bias=m2[:, :1],
                         accum_out=ss[:])
    sd = sb.tile([128, 1], F32, tag="ln_sd")
    nc.scalar.activation(out=sd[:], in_=ss[:], func=ACT.Sqrt, bias=epsc[:, :1],
                         scale=1.0 / Fd)
    rs = sb.tile([128, 1], F32, tag="ln_rs")
    nc.vector.reciprocal(out=rs[:], in_=sd[:])
    nc.vector.tensor_scalar(out=y[:], in0=y[:], scalar1=m2[:, :1], scalar2=rs[:, :1],
                            op0=ALU.add, op1=ALU.mult)
    nc.vector.tensor_tensor(out=y[:], in0=y[:], in1=g_t[:, :Fd], op=ALU.mult)
    nc.vector.tensor_tensor(out=y[:], in0=y[:], in1=be_t[:, :Fd], op=ALU.add)
    nc.vector.tensor_scalar_min(out=sq[:], in0=y[:], scalar1=0.0)
    nc.scalar.activation(out=sq[:], in_=sq[:], func=ACT.Exp)
    h = sb.tile([128, Fd], BF16, tag=tag)
    nc.vector.tensor_scalar(out=h[:], in0=y[:], scalar1=0.0, scalar2=-1.0,
                            op0=ALU.max, op1=ALU.add)
    nc.vector.tensor_tensor(out=h[:], in0=h[:], in1=sq[:], op=ALU.add)
    return h


def lrelu02(nc, sb, src_ap, shape, tag):
    """max(x, 0.2x) -> new bf16 tile of `shape`."""
    ea = sb.tile(shape, BF16, tag=tag)
    nc.vector.tensor_scalar_mul(out=ea[:], in0=src_ap, scalar1=0.2)
    nc.vector.tensor_tensor(out=ea[:], in0=src_ap, in1=ea[:], op=ALU.max)
    return ea


def build(meta):
    Tc, NTT = meta['Tc'], meta['NTT']
    Tg = [sum(r) for r in Tc]
    toff = np.concatenate([[0], np.cumsum(Tg)]).astype(int)

    nc = bacc.Bacc("TRN2", num_devices=NC)
    t_cntT4 = nc.dram_tensor("cntT4", [128, NPCP], BF16, kind="ExternalInput")
    t_ETlo = nc.dram_tensor("ETlo", [128, NPCP], BF16, kind="ExternalInput")
    t_EThi = nc.dram_tensor("EThi", [128, NPCP], BF16, kind="ExternalInput")
    t_idx = nc.dram_tensor("idx16", [128, NTT * 8], I16, kind="ExternalInput")
    t_Su8 = nc.dram_tensor("Su8", [128, NTT * 128], U8, kind="ExternalInput")
    t_STu8 = nc.dram_tensor("STu8", [128, NTT * 128], U8, kind="ExternalInput")
    t_bat = nc.dram_tensor("batchc", [128, NG], F32, kind="ExternalInput")
    cn = {}
    cshapes = dict(z1w_lo=[128, 512], z1w_hi=[128, 512], ind_lo=[128, 8],
                   ind_hi=[128, 8], W2s=[128, 2080], W3s=[128, 80],
                   b1t=[128, 512], g1t=[128, 512], be1t=[128, 512],
                   b2t=[128, 512], g2t=[128, 512], be2t=[128, 512],
                   b3t=[128, 16], g3t=[128, 16], be3t=[128, 16])
    for nm, sh in cshapes.items():
        cn[nm] = nc.dram_tensor(nm, sh, BF16, kind="ExternalInput")
    t_iota = nc.dram_tensor("iotaF50", [128, 64], F32, kind="ExternalInput")
    t_out = nc.dram_tensor("part", [G, 17], F32, kind="ExternalOutput")

    with tile.TileContext(nc) as tc:
        with tc.tile_pool(name="const", bufs=1) as cp, \
             tc.tile_pool(name="sb", bufs=2) as sb, \
             tc.tile_pool(name="gb", bufs=2) as gb, \
             tc.tile_pool(name="sgb", bufs=2) as sgb, \
             tc.tile_pool(name="ps", bufs=2, space="PSUM") as ps, \
             tc.tile_pool(name="pz", bufs=2, space="PSUM") as pzp, \
             tc.tile_pool(name="pacc", bufs=1, space="PSUM") as pacc, \
             tc.tile_pool(name="dram", bufs=1, space="DRAM") as dp:

            C = {}
            for nm, sh in cshapes.items():
                C[nm] = cp.tile(sh, BF16, tag="c_" + nm, name="c_" + nm)
                nc.sync.dma_start(out=C[nm][:], in_=cn[nm][:])
            C['iotaF50'] = cp.tile([128, 64], F32, tag="c_iota", name="c_iota")
            nc.sync.dma_start(out=C['iotaF50'][:], in_=t_iota[:])
            epsc = cp.tile([128, 1], F32, name="epsc")
            nc.vector.memset(epsc[:], EPS)
            batv = cp.tile([128, NG], F32, name="batv")
            nc.sync.dma_start(out=batv[:], in_=t_bat[:])
            arloc2 = cp.tile([128, NG * 4], BF16, name="arloc2")
            arloc3 = cp.tile([128, NG], BF16, name="arloc3")

            rec2_sh = dp.tile([NPCP, REC2], BF16)
            rec3_sh = dp.tile([NPCP, REC3], BF16)
            rec2f = [dp.tile([CSZ, REC2], BF16, addr_space="Shared", name=f"rec2f{k}")
                     for k in range(4)]
            rec3f = [dp.tile([CSZ, REC3], BF16, addr_space="Shared", name=f"rec3f{k}")
                     for k in range(4)]

            # ================= L1 + phaseA(W2) =================
            for g in range(NG):
                r0 = g * 128
                cg = sb.tile([128, 128], BF16, tag="cg")
                nc.sync.dma_start(out=cg[:], in_=t_cntT4[:, r0:r0 + 128])
                elo = sb.tile([128, 128], BF16, tag="elo")
                nc.sync.dma_start(out=elo[:], in_=t_ETlo[:, r0:r0 + 128])
                ehi = sb.tile([128, 128], BF16, tag="ehi")
                nc.sync.dma_start(out=ehi[:], in_=t_EThi[:, r0:r0 + 128])
                Mlo = sb.tile([128, 128], BF16, tag="Mlo")
                nc.vector.tensor_tensor(out=Mlo[:], in0=elo[:], in1=cg[:], op=ALU.mult)
                Mhi = sb.tile([128, 128], BF16, tag="Mhi")
                nc.vector.tensor_tensor(out=Mhi[:], in0=ehi[:], in1=cg[:], op=ALU.mult)
                pO = ps.tile([128, 512], F32, tag="pbig", space="PSUM")
                psm = ps.tile([128, 512], F32, tag="psm", space="PSUM")
                nc.tensor.matmul(out=pO[:], lhsT=Mlo[:], rhs=C['z1w_lo'][:],
                                 start=True, stop=False)
                nc.tensor.matmul(out=psm[:, 0:8], lhsT=Mlo[:], rhs=C['ind_lo'][:],
                                 start=True, stop=False)
                nc.tensor.matmul(out=pO[:], lhsT=Mhi[:], rhs=C['z1w_hi'][:],
                                 start=False, stop=True)
                nc.tensor.matmul(out=psm[:, 0:8], lhsT=Mhi[:], rhs=C['ind_hi'][:],
                                 start=False, stop=True)
                rs8 = sb.tile([128, 8], F32, tag="rs8")
                nc.vector.reciprocal(out=rs8[:], in_=psm[:, 0:8])
                y = sb.tile([128, 512], BF16, tag="y")
                nc.vector.tensor_tensor(
                    out=y[:].rearrange("p (c h) -> p c h", h=H1),
                    in0=pO[:].rearrange("p (c h) -> p c h", h=H1),
                    in1=rs8[:, None, :].to_broadcast([128, C1, H1]),
                    op=ALU.mult)
                nc.vector.tensor_tensor(out=y[:], in0=y[:], in1=C['b1t'][:], op=ALU.add)
                h1 = ln_elu(nc, sb, y, C['g1t'], C['be1t'], 512, epsc, "h1")
                z2p = pzp.tile([128, 512], F32, tag="pz", space="PSUM")
                for k in range(4):
                    hT = sb.tile([128, 128], BF16, tag="hT")
                    nc.sync.dma_start_transpose(out=hT[:], in_=h1[:, k * 128:(k + 1) * 128])
                    nc.tensor.matmul(out=z2p[:], lhsT=hT[:],
                                     rhs=C['W2s'][:, k * 520:k * 520 + 512],
                                     start=(k == 0), stop=(k == 3))
                    nc.tensor.matmul(out=psm[:, 16:24], lhsT=hT[:],
                                     rhs=C['W2s'][:, k * 520 + 512:(k + 1) * 520],
                                     start=(k == 0), stop=(k == 3))
                zs = sb.tile([128, 516], BF16, tag="zs")
                nc.vector.tensor_copy(out=zs[:, 0:512], in_=z2p[:])
                nc.vector.tensor_copy(out=zs[:, 512:516], in_=psm[:, 16:20])
                nc.vector.tensor_copy(out=arloc2[:, 4 * g:4 * g + 4], in_=psm[:, 20:24])
                nc.sync.dma_start(out=rec2_sh[r0:r0 + 128, 0:516], in_=zs[:])
                if g in (12, 24, 36, 48):
                    k = {12: 0, 24: 1, 36: 2, 48: 3}[g]
                    a, b = int(RB[k]), int(RB[k + 1])
                    nc.gpsimd.collective_compute(
                        "AllGather", ALU.bypass, replica_groups=[list(range(NC))],
                        ins=[rec2_sh[a:b, :]], outs=[rec2f[k][:, :]])

            # ================= L2 + phaseA(W3) =================
            for g in range(NG):
                r0 = g * 128
                T, o = Tg[g], int(toff[g])
                ig = sb.tile([128, T * 8], I16, tag="ig")
                nc.sync.dma_start(out=ig[:], in_=t_idx[:, o * 8:(o + T) * 8])
                Sb = sgb.tile([128, T, 128], BF16, tag="Sb")
                nc.gpsimd.dma_start(
                    out=Sb[:, :, :],
                    in_=t_Su8[:, o * 128:(o + T) * 128].rearrange("p (t j) -> p t j", j=128))
                STb = sgb.tile([128, T, 128], BF16, tag="STb")
                nc.gpsimd.dma_start(
                    out=STb[:, :, :],
                    in_=t_STu8[:, o * 128:(o + T) * 128].rearrange("p (t j) -> p t j", j=128))
                zg = sb.tile([128, 516], BF16, tag="zg")
                nc.sync.dma_start(out=zg[:], in_=rec2_sh[r0:r0 + 128, 0:516])
                Gt = gb.tile([128, T, REC2], BF16, tag="Gt")
                tcur = 0
                for k in range(4):
                    Tk = Tc[g][k]
                    if Tk:
                        nc.gpsimd.dma_gather(
                            Gt[:, tcur:tcur + Tk, :], rec2f[k][:, :],
                            ig[:, tcur * 8:(tcur + Tk) * 8], Tk * 128, Tk * 128, REC2)
                        tcur += Tk
                psm = ps.tile([128, 512], F32, tag="psm", space="PSUM")
                pAR = psm[:, 0:T * 4]
                for t in range(T):
                    nc.tensor.matmul(out=psm[:, 4 * t:4 * t + 4], lhsT=STb[:, t, :],
                                     rhs=arloc2[:, 4 * g:4 * g + 4], start=True, stop=True)
                eL = sb.tile([128, T, 4], BF16, tag="eL")
                nc.vector.tensor_tensor(
                    out=eL[:, :, :], in0=Gt[:, :, 512:516],
                    in1=pAR.rearrange("p (t h) -> p t h", h=4), op=ALU.add)
                eA = lrelu02(nc, sb, eL[:, :, :], [128, T, 4], "eA")
                nc.scalar.activation(out=Gt[:, :, 512:516], in_=eA[:, :, :], func=ACT.Exp)
                nc.vector.tensor_tensor(
                    out=Gt[:, :, 0:512].rearrange("p t (c h) -> p t c h", h=H2),
                    in0=Gt[:, :, 0:512].rearrange("p t (c h) -> p t c h", h=H2),
                    in1=Gt[:, :, 512:516][:, :, None, :].to_broadcast([128, T, C2, H2]),
                    op=ALU.mult)
                pMain = ps.tile([128, 512], F32, tag="pbig", space="PSUM")
                pS = psm[:, 96:100]
                for t in range(T):
                    nc.tensor.matmul(out=pMain[:], lhsT=Sb[:, t, :], rhs=Gt[:, t, 0:512],
                                     start=(t == 0), stop=(t == T - 1))
                    nc.tensor.matmul(out=pS, lhsT=Sb[:, t, :], rhs=Gt[:, t, 512:516],
                                     start=(t == 0), stop=(t == T - 1))
                eSl = sb.tile([128, 4], BF16, tag="eSl")
                nc.vector.tensor_tensor(out=eSl[:], in0=zg[:, 512:516],
                                        in1=arloc2[:, 4 * g:4 * g + 4], op=ALU.add)
                eSa = lrelu02(nc, sb, eSl[:], [128, 4], "eSa")
                exS = sb.tile([128, 4], BF16, tag="exS")
                nc.scalar.activation(out=exS[:], in_=eSa[:], func=ACT.Exp)
                selfc = sb.tile([128, 512], BF16, tag="selfc")
                nc.vector.tensor_tensor(
                    out=selfc[:].rearrange("p (c h) -> p c h", h=H2),
                    in0=zg[:, 0:512].rearrange("p (c h) -> p c h", h=H2),
                    in1=exS[:, None, :].to_broadcast([128, C2, H2]), op=ALU.mult)
                nc.vector.tensor_tensor(out=selfc[:], in0=pMain[:], in1=selfc[:], op=ALU.add)
                sS = sb.tile([128, 4], F32, tag="sS")
                nc.vector.tensor_tensor(out=sS[:], in0=pS, in1=exS[:], op=ALU.add)
                rS = sb.tile([128, 4], F32, tag="rS")
                nc.vector.reciprocal(out=rS[:], in_=sS[:])
                nc.vector.tensor_tensor(
                    out=selfc[:].rearrange("p (c h) -> p c h", h=H2),
                    in0=selfc[:].rearrange("p (c h) -> p c h", h=H2),
                    in1=rS[:, None, :].to_broadcast([128, C2, H2]), op=ALU.mult)
                nc.vector.tensor_tensor(out=selfc[:], in0=selfc[:], in1=C['b2t'][:], op=ALU.add)
                h2 = ln_elu(nc, sb, selfc, C['g2t'], C['be2t'], 512, epsc, "h2")
                z3p = pzp.tile([128, 20], F32, tag="pz", space="PSUM")
                for k in range(4):
                    hT = sb.tile([128, 128], BF16, tag="hT")
                    nc.sync.dma_start_transpose(out=hT[:], in_=h2[:, k * 128:(k + 1) * 128])
                    nc.tensor.matmul(out=z3p[:], lhsT=hT[:],
                                     rhs=C['W3s'][:, k * 20:(k + 1) * 20],
                                     start=(k == 0), stop=(k == 3))
                zs3 = sb.tile([128, 17], BF16, tag="zs3")
                nc.vector.tensor_copy(out=zs3[:], in_=z3p[:, 0:17])
                nc.vector.tensor_copy(out=arloc3[:, g:g + 1], in_=z3p[:, 17:18])
                nc.sync.dma_start(out=rec3_sh[r0:r0 + 128, 0:17], in_=zs3[:])
                if g in (26, 48):
                    for k in ((0, 1) if g == 26 else (2, 3)):
                        a, b = int(RB[k]), int(RB[k + 1])
                        nc.gpsimd.collective_compute(
                            "AllGather", ALU.bypass, replica_groups=[list(range(NC))],
                            ins=[rec3_sh[a:b, :]], outs=[rec3f[k][:, :]])

            # ================= L3 + pooling =================
            pPool = pacc.tile([128, 17], F32, tag="pPool", space="PSUM")
            for g in range(NG):
                r0 = g * 128
                T, o = Tg[g], int(toff[g])
                ig = sb.tile([128, T * 8], I16, tag="ig")
                nc.sync.dma_start(out=ig[:], in_=t_idx[:, o * 8:(o + T) * 8])
                Sb = sgb.tile([128, T, 128], BF16, tag="Sb")
                nc.gpsimd.dma_start(
                    out=Sb[:, :, :],
                    in_=t_Su8[:, o * 128:(o + T) * 128].rearrange("p (t j) -> p t j", j=128))
                STb = sgb.tile([128, T, 128], BF16, tag="STb")
                nc.gpsimd.dma_start(
                    out=STb[:, :, :],
                    in_=t_STu8[:, o * 128:(o + T) * 128].rearrange("p (t j) -> p t j", j=128))
                zg3 = sb.tile([128, 17], BF16, tag="zg3")
                nc.sync.dma_start(out=zg3[:], in_=rec3_sh[r0:r0 + 128, 0:17])
                Gt3 = gb.tile([128, T, REC3], BF16, tag="Gt3")
                tcur = 0
                for k in range(4):
                    Tk = Tc[g][k]
                    if Tk:
                        nc.gpsimd.dma_gather(
                            Gt3[:, tcur:tcur + Tk, :], rec3f[k][:, :],
                            ig[:, tcur * 8:(tcur + Tk) * 8], Tk * 128, Tk * 128, REC3)
                        tcur += Tk
                psm = ps.tile([128, 512], F32, tag="psm", space="PSUM")
                pAR3 = psm[:, 0:T]
                for t in range(T):
                    nc.tensor.matmul(out=psm[:, t:t + 1], lhsT=STb[:, t, :],
                                     rhs=arloc3[:, g:g + 1], start=True, stop=True)
                eL3 = sb.tile([128, T], BF16, tag="eL3")
                nc.vector.tensor_tensor(out=eL3[:], in0=Gt3[:, :, 16], in1=pAR3, op=ALU.add)
                eA3 = lrelu02(nc, sb, eL3[:], [128, T], "eA3")
                nc.scalar.activation(out=Gt3[:, :, 16], in_=eA3[:], func=ACT.Exp)
                nc.vector.tensor_tensor(
                    out=Gt3[:, :, 0:16], in0=Gt3[:, :, 0:16],
                    in1=Gt3[:, :, 16:17].to_broadcast([128, T, 16]), op=ALU.mult)
                pM3 = ps.tile([128, 16], F32, tag="pbig", space="PSUM")
                pS3 = psm[:, 96:97]
                for t in range(T):
                    nc.tensor.matmul(out=pM3[:], lhsT=Sb[:, t, :], rhs=Gt3[:, t, 0:16],
                                     start=(t == 0), stop=(t == T - 1))
                    nc.tensor.matmul(out=pS3, lhsT=Sb[:, t, :], rhs=Gt3[:, t, 16:17],
                                     start=(t == 0), stop=(t == T - 1))
                eS3 = sb.tile([128, 1], BF16, tag="eS3")
                nc.vector.tensor_tensor(out=eS3[:], in0=zg3[:, 16:17],
                                        in1=arloc3[:, g:g + 1], op=ALU.add)
                eA3s = lrelu02(nc, sb, eS3[:], [128, 1], "eA3s")
                exS3 = sb.tile([128, 1], F32, tag="exS3")
                nc.scalar.activation(out=exS3[:], in_=eA3s[:], func=ACT.Exp)
                selfc3 = sb.tile([128, 16], BF16, tag="selfc3")
                nc.vector.tensor_scalar(out=selfc3[:], in0=zg3[:, 0:16],
                                        scalar1=exS3[:, :1], scalar2=None, op0=ALU.mult)
                nc.vector.tensor_tensor(out=selfc3[:], in0=pM3[:], in1=selfc3[:], op=ALU.add)
                sS3 = sb.tile([128, 1], F32, tag="sS3")
                nc.vector.tensor_tensor(out=sS3[:], in0=pS3, in1=exS3[:], op=ALU.add)
                rS3 = sb.tile([128, 1], F32, tag="rS3")
                nc.vector.reciprocal(out=rS3[:], in_=sS3[:])
                nc.vector.tensor_scalar(out=selfc3[:], in0=selfc3[:], scalar1=rS3[:, :1],
                                        scalar2=None, op0=ALU.mult)
                nc.vector.tensor_tensor(out=selfc3[:], in0=selfc3[:], in1=C['b3t'][:], op=ALU.add)
                h3 = ln_elu(nc, sb, selfc3, C['g3t'], C['be3t'], 16, epsc, "h3")
                OB = sb.tile([128, 50], BF16, tag="OB")
                nc.vector.tensor_tensor(
                    out=OB[:], in0=C['iotaF50'][:, 0:50],
                    in1=batv[:, g:g + 1].to_broadcast([128, 50]), op=ALU.is_equal)
                h3w = sb.tile([128, 17], BF16, tag="h3w")
                nc.vector.tensor_copy(out=h3w[:, 0:16], in_=h3[:])
                nc.vector.memset(h3w[:, 16:17], 1.0)
                nc.tensor.matmul(out=pPool[:G, :], lhsT=OB[:], rhs=h3w[:],
                                 start=(g == 0), stop=(g == NG - 1))
            po = sb.tile([128, 17], F32, tag="po")
            nc.vector.tensor_copy(out=po[:G, :], in_=pPool[:G, :])
            nc.sync.dma_start(out=t_out[:, :], in_=po[:G, :])
    nc.finalize()
    return nc


# ======================= host emulation (debug) =======================
def emulate(consts, percore, meta, host):
    """Numpy mirror of the device program (f32; validates indices/layout)."""
    Tc, NTT = meta['Tc'], meta['NTT']
    Tg = [sum(r) for r in Tc]
    toff = np.concatenate([[0], np.cumsum(Tg)]).astype(int)
    Cc = {k: np.asarray(v, np.float32) for k, v in consts.items()}
    Wf2 = np.concatenate([Cc['W2s'][:, k * 520:(k + 1) * 520] for k in range(4)], 0)
    Wf3 = np.concatenate([Cc['W3s'][:, k * 20:(k + 1) * 20] for k in range(4)], 0)

    def ln(y, gt, bt):
        m = y.mean(-1, keepdims=True)
        v = ((y - m) ** 2).mean(-1, keepdims=True)
        return (y - m) / np.sqrt(v + EPS) * gt + bt

    def elu(y):
        return np.where(y > 0, y, np.exp(np.minimum(y, 0)) - 1.0)

    def lrel(x):
        return np.where(x > 0, x, 0.2 * x)

    rec2 = np.zeros((NC, NPCP, 516), np.float32)
    ar2 = np.zeros((NC, NPCP, 4), np.float32)
    for c in range(NC):
        pc = percore[c]
        cntT4 = np.asarray(pc['cntT4'], np.float32)
        ETlo = np.asarray(pc['ETlo'], np.float32)
        EThi = np.asarray(pc['EThi'], np.float32)
        for g in range(NG):
            cols = slice(g * 128, g * 128 + 128)
            Mlo = ETlo[:, cols] * cntT4[:, cols]
            Mhi = EThi[:, cols] * cntT4[:, cols]
            pO = Mlo.T @ Cc['z1w_lo'] + Mhi.T @ Cc['z1w_hi']
            s8 = Mlo.T @ Cc['ind_lo'] + Mhi.T @ Cc['ind_hi']
            y = (pO.reshape(128, C1, H1) / s8[:, None, :]).reshape(128, 512)
            y = y + Cc['b1t'][0]
            h1 = elu(ln(y, Cc['g1t'][0], Cc['be1t'][0]))
            z2 = h1 @ Wf2
            rec2[c, cols] = z2[:, 0:516]
            ar2[c, cols] = z2[:, 516:520]
    full2 = [np.zeros((CSZ, 516), np.float32) for _ in range(4)]
    for k in range(4):
        a, b = RB[k], RB[k + 1]
        for c in range(NC):
            full2[k][c * 1568:(c + 1) * 1568] = rec2[c, a:b]

    def unwrap(idx16, tbase, Tk):
        w = idx16[:16, tbase * 8:(tbase + Tk) * 8]
        return w.T.flatten().astype(np.int64)

    def layer_edges(c, pc, fulltab, arloc, zloc, Hn, Cn):
        """Returns per-core [NPCP, Hn*Cn] aggregated output (pre-bias)."""
        nzc = Hn * Cn
        out = np.zeros((NPCP, nzc), np.float32)
        idx16 = pc['idx16']
        Su8 = pc['Su8']
        STu8 = pc['STu8']
        for g in range(NG):
            T, o = Tg[g], int(toff[g])
            Gt = np.zeros((128, T, nzc + Hn), np.float32)
            tcur = 0
            for k in range(4):
                Tk = Tc[g][k]
                if Tk:
                    L = unwrap(idx16, o + tcur, Tk)
                    Gt[:, tcur:tcur + Tk] = \
                        fulltab[k][L.reshape(Tk, 128)].transpose(1, 0, 2)
                    tcur += Tk
            S = Su8[:, o * 128:(o + T) * 128].reshape(128, T, 128).astype(np.float32)
            ST = STu8[:, o * 128:(o + T) * 128].reshape(128, T, 128).astype(np.float32)
            arg = arloc[g * 128:(g + 1) * 128]          # [128, Hn]
            pAR = np.einsum('jtp,jh->pth', ST, arg)
            eL = Gt[:, :, nzc:nzc + Hn] + pAR
            EX = np.exp(lrel(eL))                        # [128, T, Hn]
            Gz = Gt[:, :, 0:nzc].reshape(128, T, Cn, Hn) * EX[:, :, None, :]
            Gz = Gz.reshape(128, T, nzc)
            pM = np.einsum('ptj,ptc->jc', S, Gz)
            pD = np.einsum('ptj,pth->jh', S, EX)
            zgz = zloc[g * 128:(g + 1) * 128]
            exS = np.exp(lrel(zgz[:, nzc:nzc + Hn] + arg))
            num = pM + (zgz[:, 0:nzc].reshape(128, Cn, Hn) * exS[:, None, :]).reshape(128, nzc)
            dden = pD + exS
            res = (num.reshape(128, Cn, Hn) / dden[:, None, :]).reshape(128, nzc)
            out[g * 128:(g + 1) * 128] = res
        return out

    rec3 = np.zeros((NC, NPCP, 17), np.float32)
    ar3 = np.zeros((NC, NPCP, 1), np.float32)
    h3s = []
    for c in range(NC):
        agg = layer_edges(c, percore[c], full2, ar2[c], rec2[c], H2, C2)
        h2r = np.zeros((NPCP, 512), np.float32)
        for g in range(NG):
            rows = slice(g * 128, g * 128 + 128)
            y = agg[rows] + Cc['b2t'][0]
            h2 = elu(ln(y, Cc['g2t'][0], Cc['be2t'][0]))
            h2r[rows] = h2
            z3 = h2 @ Wf3
            rec3[c, rows] = z3[:, 0:17]
            ar3[c, rows] = z3[:, 17:18]
    full3 = [np.zeros((CSZ, 17), np.float32) for _ in range(4)]
    for k in range(4):
        a, b = RB[k], RB[k + 1]
        for c in range(NC):
            full3[k][c * 1568:(c + 1) * 1568] = rec3[c, a:b]

    parts = np.zeros((NC, G, 17), np.float32)
    for c in range(NC):
        agg = layer_edges(c, percore[c], full3, ar3[c], rec3[c], H3, C3)
        bat = percore[c]['batchc']                       # [128, NG]
        for g in range(NG):
            rows = slice(g * 128, g * 128 + 128)
            y = agg[rows] + Cc['b3t'][0]
            h3 = elu(ln(y, Cc['g3t'][0], Cc['be3t'][0]))
            OB = (np.arange(50)[None, :] == bat[:, g][:, None]).astype(np.float32)
            h3w = np.concatenate([h3, np.ones((128, 1), np.float32)], 1)
            parts[c] += OB.T @ h3w
    tot = parts.sum(0)
    pooled = tot[:, :16] / np.maximum(tot[:, 16:17], 1.0)
    h = np.maximum(pooled @ host['fcW1'] + host['fcb1'], 0.0)
    return (h @ host['fcW2'] + host['fcb2']).astype(np.float32)


# ======================= kernel entry =======================
_CACHE = {}


def kernel(**inputs):
    consts, percore, meta, host = host_prep(inputs)
    key = tuple(tuple(r) for r in meta['Tc'])
    if key not in _CACHE:
        _CACHE[key] = build(meta)
    nc = _CACHE[key]
    in_maps = []
    for c in range(NC):
        m = dict(consts)
        m.update(percore[c])
        in_maps.append(m)
    from concourse.bass_utils import run_bass_kernel_spmd
    res = run_bass_kernel_spmd(nc, in_maps, core_ids=list(range(NC)))
    parts = np.stack([r["part"] for r in res.results])
    tot = parts.sum(axis=0)
    pooled = tot[:, :16] / np.maximum(tot[:, 16:17], 1.0)
    h = np.maximum(pooled @ host['fcW1'] + host['fcb1'], 0.0)
    return (h @ host['fcW2'] + host['fcb2']).astype(np.float32)


# revision 11
# speedup vs baseline: 1.5930x; 1.3423x over previous
"""MinamoTopoModel GAT kernel v2: bf16 + dma_gather + host-precomputed
selection matrices + chunked AllGathers.

Per 8-core SPMD, dst-sharded (6250 nodes/core, 49 groups of 128):
  L1: cnt-histogram trick, transposed (M^T on (head,tile) partitions) ->
      matmul-reductions, no per-head PE transposes.
  L2/L3: per-group fused dma_gather of src records (bf16, 640B/256B rows),
      host-precomputed S/S^T selection matrices (u8 -> bf16 cast DMA),
      channel-interleaved records so the softmax scale is one 2x DVE op,
      scatter + denominator matmuls share the S stationary.
  Records published via 4-chunk AllGathers overlapped with compute.
  Pooling -> per-core [50,17] partials; final FC on host.
"""
import numpy as np
import ml_dtypes
import concourse.bacc as bacc
import concourse.bass as bass
import concourse.mybir as mybir
import concourse.tile as tile

F32 = mybir.dt.float32
BF16 = mybir.dt.bfloat16
I16 = mybir.dt.int16
U8 = mybir.dt.uint8
AX = mybir.AxisListType
ALU = mybir.AluOpType
ACT = mybir.ActivationFunctionType
EPS = 1e-5
BF = ml_dtypes.bfloat16

N, E, G, NC = 50000, 800000, 50, 8
NPC, NG, NPCP = 6250, 49, 6272
NFULL = NC * NPCP
RB = np.array([0, 1568, 3136, 4704, 6272])
CSZ = 12544  # = 8*1568, rows per chunk table
REC2, REC3 = 640, 128
H1, C1, H2, C2, H3, C3 = 8, 64, 4, 128, 1, 16
TILE = 32


def host_prep(inputs):
    x = np.asarray(inputs['x']).astype(np.int64)
    ei = np.asarray(inputs['edge_index']).astype(np.int64)
    batch = np.asarray(inputs['batch']).astype(np.int64)
    emb = np.asarray(inputs['emb'], np.float32)
    W1 = np.asarray(inputs['W1'], np.float32)
    as1 = np.asarray(inputs['a_src1'], np.float32)
    ad1 = np.asarray(inputs['a_dst1'], np.float32)
    b1 = np.asarray(inputs['b1'], np.float32)
    g1 = np.asarray(inputs['g1'], np.float32)
    be1 = np.asarray(inputs['be1'], np.float32)
    W2 = np.asarray(inputs['W2'], np.float32)
    as2 = np.asarray(inputs['a_src2'], np.float32)
    ad2 = np.asarray(inputs['a_dst2'], np.float32)
    b2 = np.asarray(inputs['b2'], np.float32)
    g2 = np.asarray(inputs['g2'], np.float32)
    be2 = np.asarray(inputs['be2'], np.float32)
    W3 = np.asarray(inputs['W3'], np.float32)
    as3 = np.asarray(inputs['a_src3'], np.float32)
    ad3 = np.asarray(inputs['a_dst3'], np.float32)
    b3 = np.asarray(inputs['b3'], np.float32)
    g3 = np.asarray(inputs['g3'], np.float32)
    be3 = np.asarray(inputs['be3'], np.float32)

    ar512 = np.arange(512)
    perm1 = (ar512 % H1) * C1 + ar512 // H1   # interleaved col n <- orig col
    perm2 = (ar512 % H2) * C2 + ar512 // H2

    # ---- L1 tables ----
    z1 = emb @ W1                                   # [32, 512]
    z1h = z1.reshape(TILE, H1, C1)
    al1t = np.einsum('thc,hc->th', z1h, as1)
    ar1t = np.einsum('thc,hc->th', z1h, ad1)
    ee = al1t.T[None, :, :] + ar1t[:, :, None]      # [xd, h, t]
    ee = np.where(ee > 0, ee, 0.2 * ee)
    E_tab = np.exp(ee).astype(np.float32)           # [32, 8, 32]

    z1i = z1[:, perm1]
    rows = np.arange(128)
    hh, tt = rows // 32, rows % 32
    colh = ar512 % H1
    z1w_lo = np.where(colh[None, :] == hh[:, None], z1i[tt, :], 0.0).astype(np.float32)
    z1w_hi = np.where(colh[None, :] == (hh + 4)[:, None], z1i[tt, :], 0.0).astype(np.float32)
    ind_lo = (hh[:, None] == np.arange(8)[None, :]).astype(np.float32)
    ind_hi = ((hh + 4)[:, None] == np.arange(8)[None, :]).astype(np.float32)

    src_all = np.concatenate([ei[0], np.arange(N)])
    dst_all = np.concatenate([ei[1], np.arange(N)])
    cnt = np.zeros((N, TILE), np.float32)
    np.add.at(cnt, (dst_all, x[src_all]), 1.0)

    # ---- W2/W3 with interleave ----
    W2z = W2[:, perm2]
    W2r3 = W2.reshape(512, H2, C2)
    W2a = np.einsum('khc,hc->kh', W2r3, as2)
    W2r = np.einsum('khc,hc->kh', W2r3, ad2)
    Wf2 = np.concatenate([W2z, W2a, W2r], 1)[perm1, :]          # [512, 520]
    W2s = np.concatenate([Wf2[k * 128:(k + 1) * 128] for k in range(4)], 1)

    W3a = (W3.reshape(512, 16) @ as3[0])[:, None]
    W3r = (W3.reshape(512, 16) @ ad3[0])[:, None]
    Wf3 = np.concatenate([W3, W3a, W3r, np.zeros((512, 2), np.float32)], 1)[perm2, :]
    W3s = np.concatenate([Wf3[k * 128:(k + 1) * 128] for k in range(4)], 1)  # [128, 80]

    def bcast(v):
        return np.tile(v[None, :], (128, 1)).astype(np.float32)

    consts = dict(
        z1w_lo=z1w_lo, z1w_hi=z1w_hi, ind_lo=ind_lo, ind_hi=ind_hi,
        W2s=W2s, W3s=W3s,
        b1t=bcast(b1[perm1]), g1t=bcast(g1[perm1]), be1t=bcast(be1[perm1]),
        b2t=bcast(b2[perm2]), g2t=bcast(g2[perm2]), be2t=bcast(be2[perm2]),
        b3t=bcast(b3), g3t=bcast(g3), be3t=bcast(be3),
        iotaF50=np.tile(np.arange(64, dtype=np.float32), (128, 1)),
    )

    # ---- edges ----
    sz = np.diff(RB)
    off = 8 * RB[:-1]

    def grow(c, r):
        k = np.searchsorted(RB, r, side='right') - 1
        return off[k] + c * sz[k] + (r - RB[k])

    es, ed = ei[0], ei[1]
    cs, rsr = es // NPC, es % NPC
    ck = np.searchsorted(RB, rsr, side='right') - 1          # src chunk 0..3
    cidx = cs * 1568 + (rsr - RB[ck])                         # row in chunk table
    cd, rd = ed // NPC, ed % NPC
    gd, dl = rd // 128, rd % 128
    ncnt = np.zeros((NC, NG, 4), np.int64)
    np.add.at(ncnt, (cd, gd, ck), 1)
    Tc = np.ceil(ncnt.max(0) / 128).astype(int)               # [NG, 4]
    Tg = Tc.sum(1)
    toff = np.concatenate([[0], np.cumsum(Tg)]).astype(int)
    NTT = int(toff[-1])

    order = np.lexsort((cidx, ck, gd, cd))
    cidx_s, dl_s = cidx[order], dl[order]
    cd_s, gd_s, ck_s = cd[order], gd[order], ck[order]
    key = (cd_s * NG + gd_s) * 4 + ck_s
    bounds = np.searchsorted(key, np.arange(NC * NG * 4 + 1))

    percore = []
    for c in range(NC):
        idx16 = np.zeros((16, NTT * 8), np.int16)
        Su8 = np.zeros((128, NTT * 128), np.uint8)
        STu8 = np.zeros((128, NTT * 128), np.uint8)
        # filled below; converted to bf16 after the loop
        for g in range(NG):
            base4 = (c * NG + g) * 4
            tbase = int(toff[g])
            for k in range(4):
                Tk = int(Tc[g, k])
                s, e = bounds[base4 + k], bounds[base4 + k + 1]
                n = e - s
                if Tk == 0:
                    assert n == 0
                    continue
                L = np.zeros(Tk * 128, np.int64)
                L[:n] = cidx_s[s:e]
                idx16[:, tbase * 8:(tbase + Tk) * 8] = \
                    L.reshape(-1, 16).T.astype(np.int16)
                p_ = np.arange(n) % 128
                t_ = tbase + np.arange(n) // 128
                d_ = dl_s[s:e]
                Su8[p_, t_ * 128 + d_] = 1
                STu8[d_, t_ * 128 + p_] = 1
                tbase += Tk
        lo_n, hi_n = c * NPC, (c + 1) * NPC
        cntc = np.zeros((NPCP, TILE), np.float32)
        cntc[:NPC] = cnt[lo_n:hi_n]
        cntc[NPC:, 0] = 1.0
        cntT4 = np.tile(np.ascontiguousarray(cntc.T), (4, 1))    # [128, NPCP]
        ETf = np.ones((8, 32, NPCP), np.float32)
        ETf[:, :, :NPC] = np.moveaxis(E_tab[x[lo_n:hi_n]], 0, -1)
        batchc = np.full((NPCP,), 200.0, np.float32)
        batchc[:NPC] = batch[lo_n:hi_n]
        percore.append(dict(
            cntT4=cntT4.astype(BF),
            ETlo=ETf[0:4].reshape(128, NPCP).astype(BF),
            EThi=ETf[4:8].reshape(128, NPCP).astype(BF),
            idx16=np.tile(idx16, (8, 1)),
            Su8=Su8.astype(BF), STu8=STu8.astype(BF),
            batchc=np.ascontiguousarray(batchc.reshape(NG, 128).T),
        ))

    for k in consts:
        if k != 'iotaF50':
            consts[k] = consts[k].astype(BF)

    meta = dict(Tc=Tc.tolist(), NTT=NTT)
    host = dict(fcW1=np.asarray(inputs['fcW1'], np.float32),
                fcb1=np.asarray(inputs['fcb1'], np.float32),
                fcW2=np.asarray(inputs['fcW2'], np.float32),
                fcb2=np.asarray(inputs['fcb2'], np.float32))
    return consts, percore, meta, host


def ln_elu(nc, sb, y, g_t, be_t, Fd, epsc, tag, out_ap=None):
    """y [128,Fd] bf16 in SBUF -> elu(LN(y)*g+be); returns bf16 tile or writes out_ap."""
    s1 = sb.tile([128, 1], F32, tag="ln_s1")
    nc.vector.tensor_reduce(out=s1[:], in_=y[:], axis=AX.X, op=ALU.add)
    m2 = sb.tile([128, 1], F32, tag="ln_m2")
    nc.vector.tensor_scalar_mul(out=m2[:], in0=s1[:], scalar1=-1.0 / Fd)
    sq = sb.tile([128, Fd], BF16, tag="ln_sq")
    ss = sb.tile([128, 1], F32, tag="ln_ss")
    nc.scalar.activation(out=sq[:], in_=y[:], func=ACT.Square, bias=m2[:, :1],
                         accum_out=ss[:])
    sd = sb.tile([128, 1], F32, tag="ln_sd")
    nc.scalar.activation(out=sd[:], in_=ss[:], func=ACT.Sqrt, bias=epsc[:, :1],
                         scale=1.0 / Fd)
    rs = sb.tile([128, 1], F32, tag="ln_rs")
    nc.vector.reciprocal(out=rs[:], in_=sd[:])
    mrs = sb.tile([128, 1], F32, tag="ln_mrs")
    nc.vector.tensor_tensor(out=mrs[:], in0=m2[:], in1=rs[:], op=ALU.mult)
    # normalized y -> sq (ACT): (y + m2) * rs = y*rs + mrs
    nc.scalar.activation(out=sq[:], in_=y[:], func=ACT.Identity, scale=rs[:, :1],
                         bias=mrs[:, :1])
    nc.vector.tensor_tensor(out=sq[:], in0=sq[:], in1=g_t[:, :Fd], op=ALU.mult)
    nc.vector.tensor_tensor(out=sq[:], in0=sq[:], in1=be_t[:, :Fd], op=ALU.add)
    # ELU: exp(min(x,0)) = exp(-relu(-x)) on ACT; max part on DVE
    nc.scalar.activation(out=y[:], in_=sq[:], func=ACT.Relu, scale=-1.0)
    nc.scalar.activation(out=y[:], in_=y[:], func=ACT.Exp, scale=-1.0)
    if out_ap is None:
        h = sb.tile([128, Fd], BF16, tag=tag)
        out_ap = h[:]
    else:
        h = None
    nc.vector.tensor_scalar(out=out_ap, in0=sq[:], scalar1=0.0, scalar2=-1.0,
                            op0=ALU.max, op1=ALU.add)
    nc.vector.tensor_tensor(out=out_ap, in0=out_ap, in1=y[:], op=ALU.add)
    return h


def lrelu02(nc, sb, src_ap, shape, tag):
    """max(x, 0.2x) -> new bf16 tile of `shape`."""
    ea = sb.tile(shape, BF16, tag=tag)
    nc.vector.tensor_scalar_mul(out=ea[:], in0=src_ap, scalar1=0.2)
    nc.vector.tensor_tensor(out=ea[:], in0=src_ap, in1=ea[:], op=ALU.max)
    return ea


def build(meta):
    Tc, NTT = meta['Tc'], meta['NTT']
    Tg = [sum(r) for r in Tc]
    toff = np.concatenate([[0], np.cumsum(Tg)]).astype(int)

    nc = bacc.Bacc("TRN2", num_devices=NC)
    t_cntT4 = nc.dram_tensor("cntT4", [128, NPCP], BF16, kind="ExternalInput")
    t_ETlo = nc.dram_tensor("ETlo", [128, NPCP], BF16, kind="ExternalInput")
    t_EThi = nc.dram_tensor("EThi", [128, NPCP], BF16, kind="ExternalInput")
    t_idx = nc.dram_tensor("idx16", [128, NTT * 8], I16, kind="ExternalInput")
    t_Su8 = nc.dram_tensor("Su8", [128, NTT * 128], BF16, kind="ExternalInput")
    t_STu8 = nc.dram_tensor("STu8", [128, NTT * 128], BF16, kind="ExternalInput")
    t_bat = nc.dram_tensor("batchc", [128, NG], F32, kind="ExternalInput")
    cn = {}
    cshapes = dict(z1w_lo=[128, 512], z1w_hi=[128, 512], ind_lo=[128, 8],
                   ind_hi=[128, 8], W2s=[128, 2080], W3s=[128, 80],
                   b1t=[128, 512], g1t=[128, 512], be1t=[128, 512],
                   b2t=[128, 512], g2t=[128, 512], be2t=[128, 512],
                   b3t=[128, 16], g3t=[128, 16], be3t=[128, 16])
    for nm, sh in cshapes.items():
        cn[nm] = nc.dram_tensor(nm, sh, BF16, kind="ExternalInput")
    t_iota = nc.dram_tensor("iotaF50", [128, 64], F32, kind="ExternalInput")
    t_out = nc.dram_tensor("part", [G, 17], F32, kind="ExternalOutput")

    with tile.TileContext(nc) as tc:
        with tc.tile_pool(name="const", bufs=1) as cp, \
             tc.tile_pool(name="sb", bufs=3) as sb, \
             tc.tile_pool(name="gb", bufs=3) as gb, \
             tc.tile_pool(name="sgb", bufs=2) as sgb, \
             tc.tile_pool(name="ps", bufs=2, space="PSUM") as ps, \
             tc.tile_pool(name="pz", bufs=2, space="PSUM") as pzp, \
             tc.tile_pool(name="pacc", bufs=1, space="PSUM") as pacc, \
             tc.tile_pool(name="dram", bufs=1, space="DRAM") as dp:

            C = {}
            for nm, sh in cshapes.items():
                C[nm] = cp.tile(sh, BF16, tag="c_" + nm, name="c_" + nm)
                nc.sync.dma_start(out=C[nm][:], in_=cn[nm][:])
            C['iotaF50'] = cp.tile([128, 64], F32, tag="c_iota", name="c_iota")
            nc.sync.dma_start(out=C['iotaF50'][:], in_=t_iota[:])
            epsc = cp.tile([128, 1], F32, name="epsc")
            nc.vector.memset(epsc[:], EPS)
            batv = cp.tile([128, NG], F32, name="batv")
            nc.sync.dma_start(out=batv[:], in_=t_bat[:])
            arloc2 = cp.tile([128, NG * 4], BF16, name="arloc2")
            arloc3 = cp.tile([128, NG], BF16, name="arloc3")

            rec2_sh = dp.tile([NPCP, REC2], BF16)
            rec3_sh = dp.tile([NPCP, REC3], BF16)
            rec2f = [dp.tile([CSZ, REC2], BF16, addr_space="Shared", name=f"rec2f{k}")
                     for k in range(4)]
            rec3f = [dp.tile([CSZ, REC3], BF16, addr_space="Shared", name=f"rec3f{k}")
                     for k in range(4)]

            # ================= L1 + phaseA(W2) =================
            for g in range(NG):
                r0 = g * 128
                cg = sb.tile([128, 128], BF16, tag="cg")
                nc.sync.dma_start(out=cg[:], in_=t_cntT4[:, r0:r0 + 128])
                elo = sb.tile([128, 128], BF16, tag="elo")
                nc.sync.dma_start(out=elo[:], in_=t_ETlo[:, r0:r0 + 128])
                ehi = sb.tile([128, 128], BF16, tag="ehi")
                nc.sync.dma_start(out=ehi[:], in_=t_EThi[:, r0:r0 + 128])
                Mlo = sb.tile([128, 128], BF16, tag="Mlo")
                nc.vector.tensor_tensor(out=Mlo[:], in0=elo[:], in1=cg[:], op=ALU.mult)
                Mhi = sb.tile([128, 128], BF16, tag="Mhi")
                nc.vector.tensor_tensor(out=Mhi[:], in0=ehi[:], in1=cg[:], op=ALU.mult)
                pO = ps.tile([128, 512], F32, tag="pbig", space="PSUM")
                psm = ps.tile([128, 512], F32, tag="psm", space="PSUM")
                nc.tensor.matmul(out=pO[:], lhsT=Mlo[:], rhs=C['z1w_lo'][:],
                                 start=True, stop=False)
                nc.tensor.matmul(out=psm[:, 0:8], lhsT=Mlo[:], rhs=C['ind_lo'][:],
                                 start=True, stop=False)
                nc.tensor.matmul(out=pO[:], lhsT=Mhi[:], rhs=C['z1w_hi'][:],
                                 start=False, stop=True)
                nc.tensor.matmul(out=psm[:, 0:8], lhsT=Mhi[:], rhs=C['ind_hi'][:],
                                 start=False, stop=True)
                rs8 = sb.tile([128, 8], F32, tag="rs8")
                nc.vector.reciprocal(out=rs8[:], in_=psm[:, 0:8])
                y = sb.tile([128, 512], BF16, tag="y")
                nc.vector.tensor_tensor(
                    out=y[:].rearrange("p (c h) -> p c h", h=H1),
                    in0=pO[:].rearrange("p (c h) -> p c h", h=H1),
                    in1=rs8[:, None, :].to_broadcast([128, C1, H1]),
                    op=ALU.mult)
                nc.vector.tensor_tensor(out=y[:], in0=y[:], in1=C['b1t'][:], op=ALU.add)
                h1 = ln_elu(nc, sb, y, C['g1t'], C['be1t'], 512, epsc, "h1")
                z2p = pzp.tile([128, 512], F32, tag="pz", space="PSUM")
                for k in range(4):
                    hT = sb.tile([128, 128], BF16, tag="hT")
                    nc.sync.dma_start_transpose(out=hT[:], in_=h1[:, k * 128:(k + 1) * 128])
                    nc.tensor.matmul(out=z2p[:], lhsT=hT[:],
                                     rhs=C['W2s'][:, k * 520:k * 520 + 512],
                                     start=(k == 0), stop=(k == 3))
                    nc.tensor.matmul(out=psm[:, 16:24], lhsT=hT[:],
                                     rhs=C['W2s'][:, k * 520 + 512:(k + 1) * 520],
                                     start=(k == 0), stop=(k == 3))
                zs = sb.tile([128, 516], BF16, tag="zs")
                nc.vector.tensor_copy(out=zs[:, 0:512], in_=z2p[:])
                nc.vector.tensor_copy(out=zs[:, 512:516], in_=psm[:, 16:20])
                nc.vector.tensor_copy(out=arloc2[:, 4 * g:4 * g + 4], in_=psm[:, 20:24])
                nc.sync.dma_start(out=rec2_sh[r0:r0 + 128, 0:516], in_=zs[:])
                if g in (12, 24, 36, 48):
                    k = {12: 0, 24: 1, 36: 2, 48: 3}[g]
                    a, b = int(RB[k]), int(RB[k + 1])
                    nc.gpsimd.collective_compute(
                        "AllGather", ALU.bypass, replica_groups=[list(range(NC))],
                        ins=[rec2_sh[a:b, :]], outs=[rec2f[k][:, :]])

            # ================= L2 + phaseA(W3) =================
            for g in range(NG):
                r0 = g * 128
                T, o = Tg[g], int(toff[g])
                ig = sb.tile([128, T * 8], I16, tag="ig")
                nc.sync.dma_start(out=ig[:], in_=t_idx[:, o * 8:(o + T) * 8])
                Sb = sgb.tile([128, T, 128], BF16, tag="Sb")
                nc.sync.dma_start(
                    out=Sb[:, :, :],
                    in_=t_Su8[:, o * 128:(o + T) * 128].rearrange("p (t j) -> p t j", j=128))
                STb = sgb.tile([128, T, 128], BF16, tag="STb")
                nc.sync.dma_start(
                    out=STb[:, :, :],
                    in_=t_STu8[:, o * 128:(o + T) * 128].rearrange("p (t j) -> p t j", j=128))
                zg = sb.tile([128, 516], BF16, tag="zg")
                nc.sync.dma_start(out=zg[:], in_=rec2_sh[r0:r0 + 128, 0:516])
                Gt = gb.tile([128, T, REC2], BF16, tag="Gt")
                tcur = 0
                for k in range(4):
                    Tk = Tc[g][k]
                    if Tk:
                        nc.gpsimd.dma_gather(
                            Gt[:, tcur:tcur + Tk, :], rec2f[k][:, :],
                            ig[:, tcur * 8:(tcur + Tk) * 8], Tk * 128, Tk * 128, REC2)
                        tcur += Tk
                psm = ps.tile([128, 512], F32, tag="psm", space="PSUM")
                pAR = psm[:, 0:T * 4]
                for t in range(T):
                    nc.tensor.matmul(out=psm[:, 4 * t:4 * t + 4], lhsT=STb[:, t, :],
                                     rhs=arloc2[:, 4 * g:4 * g + 4], start=True, stop=True)
                eL = sb.tile([128, T * 4], BF16, tag="eL")
                nc.vector.tensor_tensor(
                    out=eL[:].rearrange("p (t h) -> p t h", h=4), in0=Gt[:, :, 512:516],
                    in1=pAR.rearrange("p (t h) -> p t h", h=4), op=ALU.add)
                eA = lrelu02(nc, sb, eL[:], [128, T * 4], "eA")
                nc.scalar.activation(
                    out=Gt[:, :, 512:516],
                    in_=eA[:].rearrange("p (t h) -> p t h", h=4), func=ACT.Exp)
                nc.vector.tensor_tensor(
                    out=Gt[:, :, 0:512].rearrange("p t (c h) -> p t c h", h=H2),
                    in0=Gt[:, :, 0:512].rearrange("p t (c h) -> p t c h", h=H2),
                    in1=Gt[:, :, 512:516][:, :, None, :].to_broadcast([128, T, C2, H2]),
                    op=ALU.mult)
                pMain = ps.tile([128, 512], F32, tag="pbig", space="PSUM")
                pS = psm[:, 96:100]
                for t in range(T):
                    nc.tensor.matmul(out=pMain[:], lhsT=Sb[:, t, :], rhs=Gt[:, t, 0:512],
                                     start=(t == 0), stop=(t == T - 1))
                    nc.tensor.matmul(out=pS, lhsT=Sb[:, t, :], rhs=Gt[:, t, 512:516],
                                     start=(t == 0), stop=(t == T - 1))
                eSl = sb.tile([128, 4], BF16, tag="eSl")
                nc.vector.tensor_tensor(out=eSl[:], in0=zg[:, 512:516],
                                        in1=arloc2[:, 4 * g:4 * g + 4], op=ALU.add)
                eSa = lrelu02(nc, sb, eSl[:], [128, 4], "eSa")
                exS = sb.tile([128, 4], BF16, tag="exS")
                nc.scalar.activation(out=exS[:], in_=eSa[:], func=ACT.Exp)
                selfc = sb.tile([128, 512], BF16, tag="selfc")
                nc.vector.tensor_tensor(
                    out=selfc[:].rearrange("p (c h) -> p c h", h=H2),
                    in0=zg[:, 0:512].rearrange("p (c h) -> p c h", h=H2),
                    in1=exS[:, None, :].to_broadcast([128, C2, H2]), op=ALU.mult)
                nc.vector.tensor_tensor(out=selfc[:], in0=pMain[:], in1=selfc[:], op=ALU.add)
                sS = sb.tile([128, 4], F32, tag="sS")
                nc.vector.tensor_tensor(out=sS[:], in0=pS, in1=exS[:], op=ALU.add)
                rS = sb.tile([128, 4], F32, tag="rS")
                nc.vector.reciprocal(out=rS[:], in_=sS[:])
                nc.vector.tensor_tensor(
                    out=selfc[:].rearrange("p (c h) -> p c h", h=H2),
                    in0=selfc[:].rearrange("p (c h) -> p c h", h=H2),
                    in1=rS[:, None, :].to_broadcast([128, C2, H2]), op=ALU.mult)
                nc.vector.tensor_tensor(out=selfc[:], in0=selfc[:], in1=C['b2t'][:], op=ALU.add)
                h2 = ln_elu(nc, sb, selfc, C['g2t'], C['be2t'], 512, epsc, "h2")
                z3p = pzp.tile([128, 20], F32, tag="pz", space="PSUM")
                for k in range(4):
                    hT = sb.tile([128, 128], BF16, tag="hT")
                    nc.sync.dma_start_transpose(out=hT[:], in_=h2[:, k * 128:(k + 1) * 128])
                    nc.tensor.matmul(out=z3p[:], lhsT=hT[:],
                                     rhs=C['W3s'][:, k * 20:(k + 1) * 20],
                                     start=(k == 0), stop=(k == 3))
                zs3 = sb.tile([128, 17], BF16, tag="zs3")
                nc.vector.tensor_copy(out=zs3[:], in_=z3p[:, 0:17])
                nc.vector.tensor_copy(out=arloc3[:, g:g + 1], in_=z3p[:, 17:18])
                nc.sync.dma_start(out=rec3_sh[r0:r0 + 128, 0:17], in_=zs3[:])
                if g in (26, 48):
                    for k in ((0, 1) if g == 26 else (2, 3)):
                        a, b = int(RB[k]), int(RB[k + 1])
                        nc.gpsimd.collective_compute(
                            "AllGather", ALU.bypass, replica_groups=[list(range(NC))],
                            ins=[rec3_sh[a:b, :]], outs=[rec3f[k][:, :]])

            # ================= L3 + pooling =================
            pPool = pacc.tile([128, 17], F32, tag="pPool", space="PSUM")
            for g in range(NG):
                r0 = g * 128
                T, o = Tg[g], int(toff[g])
                ig = sb.tile([128, T * 8], I16, tag="ig")
                nc.sync.dma_start(out=ig[:], in_=t_idx[:, o * 8:(o + T) * 8])
                Sb = sgb.tile([128, T, 128], BF16, tag="Sb")
                nc.sync.dma_start(
                    out=Sb[:, :, :],
                    in_=t_Su8[:, o * 128:(o + T) * 128].rearrange("p (t j) -> p t j", j=128))
                STb = sgb.tile([128, T, 128], BF16, tag="STb")
                nc.sync.dma_start(
                    out=STb[:, :, :],
                    in_=t_STu8[:, o * 128:(o + T) * 128].rearrange("p (t j) -> p t j", j=128))
                zg3 = sb.tile([128, 17], BF16, tag="zg3")
                nc.sync.dma_start(out=zg3[:], in_=rec3_sh[r0:r0 + 128, 0:17])
                Gt3 = gb.tile([128, T, REC3], BF16, tag="Gt3")
                tcur = 0
                for k in range(4):
                    Tk = Tc[g][k]
                    if Tk:
                        nc.gpsimd.dma_gather(
                            Gt3[:, tcur:tcur + Tk, :], rec3f[k][:, :],
                            ig[:, tcur * 8:(tcur + Tk) * 8], Tk * 128, Tk * 128, REC3)
                        tcur += Tk
                psm = ps.tile([128, 512], F32, tag="psm", space="PSUM")
                pAR3 = psm[:, 0:T]
                for t in range(T):
                    nc.tensor.matmul(out=psm[:, t:t + 1], lhsT=STb[:, t, :],
                                     rhs=arloc3[:, g:g + 1], start=True, stop=True)
                eL3 = sb.tile([128, T], BF16, tag="eL3")
                nc.vector.tensor_tensor(out=eL3[:], in0=Gt3[:, :, 16], in1=pAR3, op=ALU.add)
                eA3 = lrelu02(nc, sb, eL3[:], [128, T], "eA3")
                nc.scalar.activation(out=Gt3[:, :, 16], in_=eA3[:], func=ACT.Exp)
                nc.vector.tensor_tensor(
                    out=Gt3[:, :, 0:16], in0=Gt3[:, :, 0:16],
                    in1=Gt3[:, :, 16:17].to_broadcast([128, T, 16]), op=ALU.mult)
                pM3 = ps.tile([128, 16], F32, tag="pbig", space="PSUM")
                pS3 = psm[:, 96:97]
                for t in range(T):
                    nc.tensor.matmul(out=pM3[:], lhsT=Sb[:, t, :], rhs=Gt3[:, t, 0:16],
                                     start=(t == 0), stop=(t == T - 1))
                    nc.tensor.matmul(out=pS3, lhsT=Sb[:, t, :], rhs=Gt3[:, t, 16:17],
                                     start=(t == 0), stop=(t == T - 1))
                eS3 = sb.tile([128, 1], BF16, tag="eS3")
                nc.vector.tensor_tensor(out=eS3[:], in0=zg3[:, 16:17],
                                        in1=arloc3[:, g:g + 1], op=ALU.add)
                eA3s = lrelu02(nc, sb, eS3[:], [128, 1], "eA3s")
                exS3 = sb.tile([128, 1], F32, tag="exS3")
                nc.scalar.activation(out=exS3[:], in_=eA3s[:], func=ACT.Exp)
                selfc3 = sb.tile([128, 16], BF16, tag="selfc3")
                nc.vector.tensor_scalar(out=selfc3[:], in0=zg3[:, 0:16],
                                        scalar1=exS3[:, :1], scalar2=None, op0=ALU.mult)
                nc.vector.tensor_tensor(out=selfc3[:], in0=pM3[:], in1=selfc3[:], op=ALU.add)
                sS3 = sb.tile([128, 1], F32, tag="sS3")
                nc.vector.tensor_tensor(out=sS3[:], in0=pS3, in1=exS3[:], op=ALU.add)
                rS3 = sb.tile([128, 1], F32, tag="rS3")
                nc.vector.reciprocal(out=rS3[:], in_=sS3[:])
                nc.vector.tensor_scalar(out=selfc3[:], in0=selfc3[:], scalar1=rS3[:, :1],
                                        scalar2=None, op0=ALU.mult)
                nc.vector.tensor_tensor(out=selfc3[:], in0=selfc3[:], in1=C['b3t'][:], op=ALU.add)
                OB = sb.tile([128, 50], BF16, tag="OB")
                nc.vector.tensor_tensor(
                    out=OB[:], in0=C['iotaF50'][:, 0:50],
                    in1=batv[:, g:g + 1].to_broadcast([128, 50]), op=ALU.is_equal)
                h3w = sb.tile([128, 17], BF16, tag="h3w")
                ln_elu(nc, sb, selfc3, C['g3t'], C['be3t'], 16, epsc, "h3",
                       out_ap=h3w[:, 0:16])
                nc.vector.memset(h3w[:, 16:17], 1.0)
                nc.tensor.matmul(out=pPool[:G, :], lhsT=OB[:], rhs=h3w[:],
                                 start=(g == 0), stop=(g == NG - 1))
            po = sb.tile([128, 17], F32, tag="po")
            nc.vector.tensor_copy(out=po[:G, :], in_=pPool[:G, :])
            nc.sync.dma_start(out=t_out[:, :], in_=po[:G, :])
    nc.finalize()
    return nc


# ======================= host emulation (debug) =======================
def emulate(consts, percore, meta, host):
    """Numpy mirror of the device program (f32; validates indices/layout)."""
    Tc, NTT = meta['Tc'], meta['NTT']
    Tg = [sum(r) for r in Tc]
    toff = np.concatenate([[0], np.cumsum(Tg)]).astype(int)
    Cc = {k: np.asarray(v, np.float32) for k, v in consts.items()}
    Wf2 = np.concatenate([Cc['W2s'][:, k * 520:(k + 1) * 520] for k in range(4)], 0)
    Wf3 = np.concatenate([Cc['W3s'][:, k * 20:(k + 1) * 20] for k in range(4)], 0)

    def ln(y, gt, bt):
        m = y.mean(-1, keepdims=True)
        v = ((y - m) ** 2).mean(-1, keepdims=True)
        return (y - m) / np.sqrt(v + EPS) * gt + bt

    def elu(y):
        return np.where(y > 0, y, np.exp(np.minimum(y, 0)) - 1.0)

    def lrel(x):
        return np.where(x > 0, x, 0.2 * x)

    rec2 = np.zeros((NC, NPCP, 516), np.float32)
    ar2 = np.zeros((NC, NPCP, 4), np.float32)
    for c in range(NC):
        pc = percore[c]
        cntT4 = np.asarray(pc['cntT4'], np.float32)
        ETlo = np.asarray(pc['ETlo'], np.float32)
        EThi = np.asarray(pc['EThi'], np.float32)
        for g in range(NG):
            cols = slice(g * 128, g * 128 + 128)
            Mlo = ETlo[:, cols] * cntT4[:, cols]
            Mhi = EThi[:, cols] * cntT4[:, cols]
            pO = Mlo.T @ Cc['z1w_lo'] + Mhi.T @ Cc['z1w_hi']
            s8 = Mlo.T @ Cc['ind_lo'] + Mhi.T @ Cc['ind_hi']
            y = (pO.reshape(128, C1, H1) / s8[:, None, :]).reshape(128, 512)
            y = y + Cc['b1t'][0]
            h1 = elu(ln(y, Cc['g1t'][0], Cc['be1t'][0]))
            z2 = h1 @ Wf2
            rec2[c, cols] = z2[:, 0:516]
            ar2[c, cols] = z2[:, 516:520]
    full2 = [np.zeros((CSZ, 516), np.float32) for _ in range(4)]
    for k in range(4):
        a, b = RB[k], RB[k + 1]
        for c in range(NC):
            full2[k][c * 1568:(c + 1) * 1568] = rec2[c, a:b]

    def unwrap(idx16, tbase, Tk):
        w = idx16[:16, tbase * 8:(tbase + Tk) * 8]
        return w.T.flatten().astype(np.int64)

    def layer_edges(c, pc, fulltab, arloc, zloc, Hn, Cn):
        """Returns per-core [NPCP, Hn*Cn] aggregated output (pre-bias)."""
        nzc = Hn * Cn
        out = np.zeros((NPCP, nzc), np.float32)
        idx16 = pc['idx16']
        Su8 = pc['Su8']
        STu8 = pc['STu8']
        for g in range(NG):
            T, o = Tg[g], int(toff[g])
            Gt = np.zeros((128, T, nzc + Hn), np.float32)
            tcur = 0
            for k in range(4):
                Tk = Tc[g][k]
                if Tk:
                    L = unwrap(idx16, o + tcur, Tk)
                    Gt[:, tcur:tcur + Tk] = \
                        fulltab[k][L.reshape(Tk, 128)].transpose(1, 0, 2)
                    tcur += Tk
            S = Su8[:, o * 128:(o + T) * 128].reshape(128, T, 128).astype(np.float32)
            ST = STu8[:, o * 128:(o + T) * 128].reshape(128, T, 128).astype(np.float32)
            arg = arloc[g * 128:(g + 1) * 128]          # [128, Hn]
            pAR = np.einsum('jtp,jh->pth', ST, arg)
            eL = Gt[:, :, nzc:nzc + Hn] + pAR
            EX = np.exp(lrel(eL))                        # [128, T, Hn]
            Gz = Gt[:, :, 0:nzc].reshape(128, T, Cn, Hn) * EX[:, :, None, :]
            Gz = Gz.reshape(128, T, nzc)
            pM = np.einsum('ptj,ptc->jc', S, Gz)
            pD = np.einsum('ptj,pth->jh', S, EX)
            zgz = zloc[g * 128:(g + 1) * 128]
            exS = np.exp(lrel(zgz[:, nzc:nzc + Hn] + arg))
            num = pM + (zgz[:, 0:nzc].reshape(128, Cn, Hn) * exS[:, None, :]).reshape(128, nzc)
            dden = pD + exS
            res = (num.reshape(128, Cn, Hn) / dden[:, None, :]).reshape(128, nzc)
            out[g * 128:(g + 1) * 128] = res
        return out

    rec3 = np.zeros((NC, NPCP, 17), np.float32)
    ar3 = np.zeros((NC, NPCP, 1), np.float32)
    h3s = []
    for c in range(NC):
        agg = layer_edges(c, percore[c], full2, ar2[c], rec2[c], H2, C2)
        h2r = np.zeros((NPCP, 512), np.float32)
        for g in range(NG):
            rows = slice(g * 128, g * 128 + 128)
            y = agg[rows] + Cc['b2t'][0]
            h2 = elu(ln(y, Cc['g2t'][0], Cc['be2t'][0]))
            h2r[rows] = h2
            z3 = h2 @ Wf3
            rec3[c, rows] = z3[:, 0:17]
            ar3[c, rows] = z3[:, 17:18]
    full3 = [np.zeros((CSZ, 17), np.float32) for _ in range(4)]
    for k in range(4):
        a, b = RB[k], RB[k + 1]
        for c in range(NC):
            full3[k][c * 1568:(c + 1) * 1568] = rec3[c, a:b]

    parts = np.zeros((NC, G, 17), np.float32)
    for c in range(NC):
        agg = layer_edges(c, percore[c], full3, ar3[c], rec3[c], H3, C3)
        bat = percore[c]['batchc']                       # [128, NG]
        for g in range(NG):
            rows = slice(g * 128, g * 128 + 128)
            y = agg[rows] + Cc['b3t'][0]
            h3 = elu(ln(y, Cc['g3t'][0], Cc['be3t'][0]))
            OB = (np.arange(50)[None, :] == bat[:, g][:, None]).astype(np.float32)
            h3w = np.concatenate([h3, np.ones((128, 1), np.float32)], 1)
            parts[c] += OB.T @ h3w
    tot = parts.sum(0)
    pooled = tot[:, :16] / np.maximum(tot[:, 16:17], 1.0)
    h = np.maximum(pooled @ host['fcW1'] + host['fcb1'], 0.0)
    return (h @ host['fcW2'] + host['fcb2']).astype(np.float32)


# ======================= kernel entry =======================
_CACHE = {}


def kernel(**inputs):
    consts, percore, meta, host = host_prep(inputs)
    key = tuple(tuple(r) for r in meta['Tc'])
    if key not in _CACHE:
        _CACHE[key] = build(meta)
    nc = _CACHE[key]
    in_maps = []
    for c in range(NC):
        m = dict(consts)
        m.update(percore[c])
        in_maps.append(m)
    from concourse.bass_utils import run_bass_kernel_spmd
    res = run_bass_kernel_spmd(nc, in_maps, core_ids=list(range(NC)))
    parts = np.stack([r["part"] for r in res.results])
    tot = parts.sum(axis=0)
    pooled = tot[:, :16] / np.maximum(tot[:, 16:17], 1.0)
    h = np.maximum(pooled @ host['fcW1'] + host['fcb1'], 0.0)
    return (h @ host['fcW2'] + host['fcb2']).astype(np.float32)


# revision 13
# speedup vs baseline: 1.5951x; 1.0014x over previous
"""MinamoTopoModel GAT kernel v2: bf16 + dma_gather + host-precomputed
selection matrices + chunked AllGathers.

Per 8-core SPMD, dst-sharded (6250 nodes/core, 49 groups of 128):
  L1: cnt-histogram trick, transposed (M^T on (head,tile) partitions) ->
      matmul-reductions, no per-head PE transposes.
  L2/L3: per-group fused dma_gather of src records (bf16, 640B/256B rows),
      host-precomputed S/S^T selection matrices (u8 -> bf16 cast DMA),
      channel-interleaved records so the softmax scale is one 2x DVE op,
      scatter + denominator matmuls share the S stationary.
  Records published via 4-chunk AllGathers overlapped with compute.
  Pooling -> per-core [50,17] partials; final FC on host.
"""
import numpy as np
import ml_dtypes
import concourse.bacc as bacc
import concourse.bass as bass
import concourse.mybir as mybir
import concourse.tile as tile

F32 = mybir.dt.float32
BF16 = mybir.dt.bfloat16
I16 = mybir.dt.int16
U8 = mybir.dt.uint8
AX = mybir.AxisListType
ALU = mybir.AluOpType
ACT = mybir.ActivationFunctionType
EPS = 1e-5
BF = ml_dtypes.bfloat16

N, E, G, NC = 50000, 800000, 50, 8
NPC, NG, NPCP = 6250, 49, 6272
NFULL = NC * NPCP
RB = np.array([0, 1568, 3136, 4704, 6272])
CSZ = 12544  # = 8*1568, rows per chunk table
REC2, REC3 = 640, 128
H1, C1, H2, C2, H3, C3 = 8, 64, 4, 128, 1, 16
TILE = 32


def host_prep(inputs):
    x = np.asarray(inputs['x']).astype(np.int64)
    ei = np.asarray(inputs['edge_index']).astype(np.int64)
    batch = np.asarray(inputs['batch']).astype(np.int64)
    emb = np.asarray(inputs['emb'], np.float32)
    W1 = np.asarray(inputs['W1'], np.float32)
    as1 = np.asarray(inputs['a_src1'], np.float32)
    ad1 = np.asarray(inputs['a_dst1'], np.float32)
    b1 = np.asarray(inputs['b1'], np.float32)
    g1 = np.asarray(inputs['g1'], np.float32)
    be1 = np.asarray(inputs['be1'], np.float32)
    W2 = np.asarray(inputs['W2'], np.float32)
    as2 = np.asarray(inputs['a_src2'], np.float32)
    ad2 = np.asarray(inputs['a_dst2'], np.float32)
    b2 = np.asarray(inputs['b2'], np.float32)
    g2 = np.asarray(inputs['g2'], np.float32)
    be2 = np.asarray(inputs['be2'], np.float32)
    W3 = np.asarray(inputs['W3'], np.float32)
    as3 = np.asarray(inputs['a_src3'], np.float32)
    ad3 = np.asarray(inputs['a_dst3'], np.float32)
    b3 = np.asarray(inputs['b3'], np.float32)
    g3 = np.asarray(inputs['g3'], np.float32)
    be3 = np.asarray(inputs['be3'], np.float32)

    ar512 = np.arange(512)
    perm1 = (ar512 % H1) * C1 + ar512 // H1   # interleaved col n <- orig col
    perm2 = (ar512 % H2) * C2 + ar512 // H2

    # ---- L1 tables ----
    z1 = emb @ W1                                   # [32, 512]
    z1h = z1.reshape(TILE, H1, C1)
    al1t = np.einsum('thc,hc->th', z1h, as1)
    ar1t = np.einsum('thc,hc->th', z1h, ad1)
    ee = al1t.T[None, :, :] + ar1t[:, :, None]      # [xd, h, t]
    ee = np.where(ee > 0, ee, 0.2 * ee)
    E_tab = np.exp(ee).astype(np.float32)           # [32, 8, 32]

    z1i = z1[:, perm1]
    rows = np.arange(128)
    hh, tt = rows // 32, rows % 32
    colh = ar512 % H1
    z1w_lo = np.where(colh[None, :] == hh[:, None], z1i[tt, :], 0.0).astype(np.float32)
    z1w_hi = np.where(colh[None, :] == (hh + 4)[:, None], z1i[tt, :], 0.0).astype(np.float32)
    ind_lo = (hh[:, None] == np.arange(8)[None, :]).astype(np.float32)
    ind_hi = ((hh + 4)[:, None] == np.arange(8)[None, :]).astype(np.float32)

    src_all = np.concatenate([ei[0], np.arange(N)])
    dst_all = np.concatenate([ei[1], np.arange(N)])
    cnt = np.zeros((N, TILE), np.float32)
    np.add.at(cnt, (dst_all, x[src_all]), 1.0)

    # ---- W2/W3 with interleave ----
    W2z = W2[:, perm2]
    W2r3 = W2.reshape(512, H2, C2)
    W2a = np.einsum('khc,hc->kh', W2r3, as2)
    W2r = np.einsum('khc,hc->kh', W2r3, ad2)
    Wf2 = np.concatenate([W2z, W2a, W2r], 1)[perm1, :]          # [512, 520]
    W2s = np.concatenate([Wf2[k * 128:(k + 1) * 128] for k in range(4)], 1)

    W3a = (W3.reshape(512, 16) @ as3[0])[:, None]
    W3r = (W3.reshape(512, 16) @ ad3[0])[:, None]
    Wf3 = np.concatenate([W3, W3a, W3r, np.zeros((512, 2), np.float32)], 1)[perm2, :]
    W3s = np.concatenate([Wf3[k * 128:(k + 1) * 128] for k in range(4)], 1)  # [128, 80]

    def bcast(v):
        return np.tile(v[None, :], (128, 1)).astype(np.float32)

    consts = dict(
        z1w_lo=z1w_lo, z1w_hi=z1w_hi, ind_lo=ind_lo, ind_hi=ind_hi,
        W2s=W2s, W3s=W3s,
        b1t=bcast(b1[perm1]), g1t=bcast(g1[perm1]), be1t=bcast(be1[perm1]),
        b2t=bcast(b2[perm2]), g2t=bcast(g2[perm2]), be2t=bcast(be2[perm2]),
        b3t=bcast(b3), g3t=bcast(g3), be3t=bcast(be3),
        iotaF50=np.tile(np.arange(64, dtype=np.float32), (128, 1)),
    )

    # ---- edges ----
    sz = np.diff(RB)
    off = 8 * RB[:-1]

    def grow(c, r):
        k = np.searchsorted(RB, r, side='right') - 1
        return off[k] + c * sz[k] + (r - RB[k])

    es, ed = ei[0], ei[1]
    cs, rsr = es // NPC, es % NPC
    ck = np.searchsorted(RB, rsr, side='right') - 1          # src chunk 0..3
    cidx = cs * 1568 + (rsr - RB[ck])                         # row in chunk table
    cd, rd = ed // NPC, ed % NPC
    gd, dl = rd // 128, rd % 128
    ncnt = np.zeros((NC, NG, 4), np.int64)
    np.add.at(ncnt, (cd, gd, ck), 1)
    Tc = np.ceil(ncnt.max(0) / 128).astype(int)               # [NG, 4]
    Tg = Tc.sum(1)
    toff = np.concatenate([[0], np.cumsum(Tg)]).astype(int)
    NTT = int(toff[-1])

    order = np.lexsort((cidx, ck, gd, cd))
    cidx_s, dl_s = cidx[order], dl[order]
    cd_s, gd_s, ck_s = cd[order], gd[order], ck[order]
    key = (cd_s * NG + gd_s) * 4 + ck_s
    bounds = np.searchsorted(key, np.arange(NC * NG * 4 + 1))

    percore = []
    for c in range(NC):
        idx16 = np.zeros((16, NTT * 8), np.int16)
        Su8 = np.zeros((128, NTT * 128), np.uint8)
        STu8 = np.zeros((128, NTT * 128), np.uint8)
        # filled below; converted to bf16 after the loop
        for g in range(NG):
            base4 = (c * NG + g) * 4
            tbase = int(toff[g])
            for k in range(4):
                Tk = int(Tc[g, k])
                s, e = bounds[base4 + k], bounds[base4 + k + 1]
                n = e - s
                if Tk == 0:
                    assert n == 0
                    continue
                L = np.zeros(Tk * 128, np.int64)
                L[:n] = cidx_s[s:e]
                idx16[:, tbase * 8:(tbase + Tk) * 8] = \
                    L.reshape(-1, 16).T.astype(np.int16)
                p_ = np.arange(n) % 128
                t_ = tbase + np.arange(n) // 128
                d_ = dl_s[s:e]
                Su8[p_, t_ * 128 + d_] = 1
                STu8[d_, t_ * 128 + p_] = 1
                tbase += Tk
        lo_n, hi_n = c * NPC, (c + 1) * NPC
        cntc = np.zeros((NPCP, TILE), np.float32)
        cntc[:NPC] = cnt[lo_n:hi_n]
        cntc[NPC:, 0] = 1.0
        cntT4 = np.tile(np.ascontiguousarray(cntc.T), (4, 1))    # [128, NPCP]
        ETf = np.ones((8, 32, NPCP), np.float32)
        ETf[:, :, :NPC] = np.moveaxis(E_tab[x[lo_n:hi_n]], 0, -1)
        batchc = np.full((NPCP,), 200.0, np.float32)
        batchc[:NPC] = batch[lo_n:hi_n]
        percore.append(dict(
            cntT4=cntT4.astype(BF),
            ETlo=ETf[0:4].reshape(128, NPCP).astype(BF),
            EThi=ETf[4:8].reshape(128, NPCP).astype(BF),
            idx16=np.tile(idx16, (8, 1)),
            Su8=Su8.astype(BF), STu8=STu8.astype(BF),
            batchc=np.ascontiguousarray(batchc.reshape(NG, 128).T),
        ))

    for k in consts:
        if k != 'iotaF50':
            consts[k] = consts[k].astype(BF)

    meta = dict(Tc=Tc.tolist(), NTT=NTT)
    host = dict(fcW1=np.asarray(inputs['fcW1'], np.float32),
                fcb1=np.asarray(inputs['fcb1'], np.float32),
                fcW2=np.asarray(inputs['fcW2'], np.float32),
                fcb2=np.asarray(inputs['fcb2'], np.float32))
    return consts, percore, meta, host


def ln_elu(nc, sb, y, g_t, be_t, Fd, epsc, tag, out_ap=None):
    """y [128,Fd] bf16 in SBUF -> elu(LN(y)*g+be); returns bf16 tile or writes out_ap."""
    s1 = sb.tile([128, 1], F32, tag="ln_s1")
    nc.vector.tensor_reduce(out=s1[:], in_=y[:], axis=AX.X, op=ALU.add)
    m2 = sb.tile([128, 1], F32, tag="ln_m2")
    nc.vector.tensor_scalar_mul(out=m2[:], in0=s1[:], scalar1=-1.0 / Fd)
    sq = sb.tile([128, Fd], BF16, tag="ln_sq")
    ss = sb.tile([128, 1], F32, tag="ln_ss")
    nc.scalar.activation(out=sq[:], in_=y[:], func=ACT.Square, bias=m2[:, :1],
                         accum_out=ss[:])
    sd = sb.tile([128, 1], F32, tag="ln_sd")
    nc.scalar.activation(out=sd[:], in_=ss[:], func=ACT.Sqrt, bias=epsc[:, :1],
                         scale=1.0 / Fd)
    rs = sb.tile([128, 1], F32, tag="ln_rs")
    nc.vector.reciprocal(out=rs[:], in_=sd[:])
    mrs = sb.tile([128, 1], F32, tag="ln_mrs")
    nc.vector.tensor_tensor(out=mrs[:], in0=m2[:], in1=rs[:], op=ALU.mult)
    # normalized y -> sq (ACT): (y + m2) * rs = y*rs + mrs
    nc.scalar.activation(out=sq[:], in_=y[:], func=ACT.Identity, scale=rs[:, :1],
                         bias=mrs[:, :1])
    nc.vector.tensor_tensor(out=sq[:], in0=sq[:], in1=g_t[:, :Fd], op=ALU.mult)
    nc.vector.tensor_tensor(out=sq[:], in0=sq[:], in1=be_t[:, :Fd], op=ALU.add)
    # ELU: exp(min(x,0)) = exp(-relu(-x)) on ACT; max part on DVE
    nc.scalar.activation(out=y[:], in_=sq[:], func=ACT.Relu, scale=-1.0)
    nc.scalar.activation(out=y[:], in_=y[:], func=ACT.Exp, scale=-1.0)
    if out_ap is None:
        h = sb.tile([128, Fd], BF16, tag=tag)
        out_ap = h[:]
    else:
        h = None
    nc.vector.tensor_scalar(out=out_ap, in0=sq[:], scalar1=0.0, scalar2=-1.0,
                            op0=ALU.max, op1=ALU.add)
    nc.vector.tensor_tensor(out=out_ap, in0=out_ap, in1=y[:], op=ALU.add)
    return h


def lrelu02(nc, sb, src_ap, shape, tag):
    """max(x, 0.2x) -> new bf16 tile of `shape`."""
    ea = sb.tile(shape, BF16, tag=tag)
    nc.vector.tensor_scalar_mul(out=ea[:], in0=src_ap, scalar1=0.2)
    nc.vector.tensor_tensor(out=ea[:], in0=src_ap, in1=ea[:], op=ALU.max)
    return ea


def build(meta):
    Tc, NTT = meta['Tc'], meta['NTT']
    Tg = [sum(r) for r in Tc]
    toff = np.concatenate([[0], np.cumsum(Tg)]).astype(int)

    nc = bacc.Bacc("TRN2", num_devices=NC)
    t_cntT4 = nc.dram_tensor("cntT4", [128, NPCP], BF16, kind="ExternalInput")
    t_ETlo = nc.dram_tensor("ETlo", [128, NPCP], BF16, kind="ExternalInput")
    t_EThi = nc.dram_tensor("EThi", [128, NPCP], BF16, kind="ExternalInput")
    t_idx = nc.dram_tensor("idx16", [128, NTT * 8], I16, kind="ExternalInput")
    t_Su8 = nc.dram_tensor("Su8", [128, NTT * 128], BF16, kind="ExternalInput")
    t_STu8 = nc.dram_tensor("STu8", [128, NTT * 128], BF16, kind="ExternalInput")
    t_bat = nc.dram_tensor("batchc", [128, NG], F32, kind="ExternalInput")
    cn = {}
    cshapes = dict(z1w_lo=[128, 512], z1w_hi=[128, 512], ind_lo=[128, 8],
                   ind_hi=[128, 8], W2s=[128, 2080], W3s=[128, 80],
                   b1t=[128, 512], g1t=[128, 512], be1t=[128, 512],
                   b2t=[128, 512], g2t=[128, 512], be2t=[128, 512],
                   b3t=[128, 16], g3t=[128, 16], be3t=[128, 16])
    for nm, sh in cshapes.items():
        cn[nm] = nc.dram_tensor(nm, sh, BF16, kind="ExternalInput")
    t_iota = nc.dram_tensor("iotaF50", [128, 64], F32, kind="ExternalInput")
    t_out = nc.dram_tensor("part", [G, 17], F32, kind="ExternalOutput")

    with tile.TileContext(nc) as tc:
        with tc.tile_pool(name="const", bufs=1) as cp, \
             tc.tile_pool(name="sb", bufs=3) as sb, \
             tc.tile_pool(name="gb", bufs=3) as gb, \
             tc.tile_pool(name="sgb", bufs=2) as sgb, \
             tc.tile_pool(name="ps", bufs=2, space="PSUM") as ps, \
             tc.tile_pool(name="pz", bufs=2, space="PSUM") as pzp, \
             tc.tile_pool(name="pacc", bufs=1, space="PSUM") as pacc, \
             tc.tile_pool(name="dram", bufs=1, space="DRAM") as dp:

            C = {}
            for nm, sh in cshapes.items():
                C[nm] = cp.tile(sh, BF16, tag="c_" + nm, name="c_" + nm)
                nc.sync.dma_start(out=C[nm][:], in_=cn[nm][:])
            C['iotaF50'] = cp.tile([128, 64], F32, tag="c_iota", name="c_iota")
            nc.sync.dma_start(out=C['iotaF50'][:], in_=t_iota[:])
            epsc = cp.tile([128, 1], F32, name="epsc")
            nc.vector.memset(epsc[:], EPS)
            batv = cp.tile([128, NG], F32, name="batv")
            nc.sync.dma_start(out=batv[:], in_=t_bat[:])
            arloc2 = cp.tile([128, NG * 4], BF16, name="arloc2")
            arloc3 = cp.tile([128, NG], BF16, name="arloc3")

            rec2_sh = dp.tile([NPCP, REC2], BF16)
            rec3_sh = dp.tile([NPCP, REC3], BF16)
            rec2f = [dp.tile([CSZ, REC2], BF16, addr_space="Shared", name=f"rec2f{k}")
                     for k in range(4)]
            rec3f = [dp.tile([CSZ, REC3], BF16, addr_space="Shared", name=f"rec3f{k}")
                     for k in range(4)]

            # ================= L1 + phaseA(W2) =================
            for g in range(NG):
                r0 = g * 128
                cg = sb.tile([128, 128], BF16, tag="cg")
                nc.sync.dma_start(out=cg[:], in_=t_cntT4[:, r0:r0 + 128])
                elo = sb.tile([128, 128], BF16, tag="elo")
                nc.sync.dma_start(out=elo[:], in_=t_ETlo[:, r0:r0 + 128])
                ehi = sb.tile([128, 128], BF16, tag="ehi")
                nc.sync.dma_start(out=ehi[:], in_=t_EThi[:, r0:r0 + 128])
                Mlo = sb.tile([128, 128], BF16, tag="Mlo")
                nc.vector.tensor_tensor(out=Mlo[:], in0=elo[:], in1=cg[:], op=ALU.mult)
                Mhi = sb.tile([128, 128], BF16, tag="Mhi")
                nc.vector.tensor_tensor(out=Mhi[:], in0=ehi[:], in1=cg[:], op=ALU.mult)
                pO = ps.tile([128, 512], F32, tag="pbig", space="PSUM")
                psm = ps.tile([128, 512], F32, tag="psm", space="PSUM")
                nc.tensor.matmul(out=pO[:], lhsT=Mlo[:], rhs=C['z1w_lo'][:],
                                 start=True, stop=False)
                nc.tensor.matmul(out=psm[:, 0:8], lhsT=Mlo[:], rhs=C['ind_lo'][:],
                                 start=True, stop=False)
                nc.tensor.matmul(out=pO[:], lhsT=Mhi[:], rhs=C['z1w_hi'][:],
                                 start=False, stop=True)
                nc.tensor.matmul(out=psm[:, 0:8], lhsT=Mhi[:], rhs=C['ind_hi'][:],
                                 start=False, stop=True)
                rs8 = sb.tile([128, 8], F32, tag="rs8")
                nc.vector.reciprocal(out=rs8[:], in_=psm[:, 0:8])
                y = sb.tile([128, 512], BF16, tag="y")
                nc.vector.tensor_tensor(
                    out=y[:].rearrange("p (c h) -> p c h", h=H1),
                    in0=pO[:].rearrange("p (c h) -> p c h", h=H1),
                    in1=rs8[:, None, :].to_broadcast([128, C1, H1]),
                    op=ALU.mult)
                nc.vector.tensor_tensor(out=y[:], in0=y[:], in1=C['b1t'][:], op=ALU.add)
                h1 = ln_elu(nc, sb, y, C['g1t'], C['be1t'], 512, epsc, "h1")
                z2p = pzp.tile([128, 512], F32, tag="pz", space="PSUM")
                for k in range(4):
                    hT = sb.tile([128, 128], BF16, tag="hT")
                    nc.sync.dma_start_transpose(out=hT[:], in_=h1[:, k * 128:(k + 1) * 128])
                    nc.tensor.matmul(out=z2p[:], lhsT=hT[:],
                                     rhs=C['W2s'][:, k * 520:k * 520 + 512],
                                     start=(k == 0), stop=(k == 3))
                    nc.tensor.matmul(out=psm[:, 16:24], lhsT=hT[:],
                                     rhs=C['W2s'][:, k * 520 + 512:(k + 1) * 520],
                                     start=(k == 0), stop=(k == 3))
                zs = sb.tile([128, 516], BF16, tag="zs")
                nc.vector.tensor_copy(out=zs[:, 0:512], in_=z2p[:])
                nc.vector.tensor_copy(out=zs[:, 512:516], in_=psm[:, 16:20])
                nc.vector.tensor_copy(out=arloc2[:, 4 * g:4 * g + 4], in_=psm[:, 20:24])
                nc.sync.dma_start(out=rec2_sh[r0:r0 + 128, 0:516], in_=zs[:])
                if g in (12, 24, 36, 48):
                    k = {12: 0, 24: 1, 36: 2, 48: 3}[g]
                    a, b = int(RB[k]), int(RB[k + 1])
                    nc.gpsimd.collective_compute(
                        "AllGather", ALU.bypass, replica_groups=[list(range(NC))],
                        ins=[rec2_sh[a:b, :]], outs=[rec2f[k][:, :]])

            # ================= L2 + phaseA(W3) =================
            for g in range(NG):
                r0 = g * 128
                T, o = Tg[g], int(toff[g])
                ig = sb.tile([128, T * 8], I16, tag="ig")
                nc.sync.dma_start(out=ig[:], in_=t_idx[:, o * 8:(o + T) * 8])
                Sb = sgb.tile([128, T, 128], BF16, tag="Sb")
                nc.sync.dma_start(
                    out=Sb[:, :, :],
                    in_=t_Su8[:, o * 128:(o + T) * 128].rearrange("p (t j) -> p t j", j=128))
                STb = sgb.tile([128, T, 128], BF16, tag="STb")
                nc.sync.dma_start(
                    out=STb[:, :, :],
                    in_=t_STu8[:, o * 128:(o + T) * 128].rearrange("p (t j) -> p t j", j=128))
                zg = sb.tile([128, 516], BF16, tag="zg")
                nc.sync.dma_start(out=zg[:], in_=rec2_sh[r0:r0 + 128, 0:516])
                Gt = gb.tile([128, T, REC2], BF16, tag="Gt")
                tcur = 0
                for k in range(4):
                    Tk = Tc[g][k]
                    if Tk:
                        nc.gpsimd.dma_gather(
                            Gt[:, tcur:tcur + Tk, :], rec2f[k][:, :],
                            ig[:, tcur * 8:(tcur + Tk) * 8], Tk * 128, Tk * 128, REC2)
                        tcur += Tk
                psm = ps.tile([128, 512], F32, tag="psm", space="PSUM")
                pAR = psm[:, 0:T * 4]
                for t in range(T):
                    nc.tensor.matmul(out=psm[:, 4 * t:4 * t + 4], lhsT=STb[:, t, :],
                                     rhs=arloc2[:, 4 * g:4 * g + 4], start=True, stop=True)
                eL = sb.tile([128, T * 4], BF16, tag="eL")
                nc.vector.tensor_tensor(
                    out=eL[:].rearrange("p (t h) -> p t h", h=4), in0=Gt[:, :, 512:516],
                    in1=pAR.rearrange("p (t h) -> p t h", h=4), op=ALU.add)
                eA = lrelu02(nc, sb, eL[:], [128, T * 4], "eA")
                nc.scalar.activation(
                    out=Gt[:, :, 512:516],
                    in_=eA[:].rearrange("p (t h) -> p t h", h=4), func=ACT.Exp)
                nc.vector.tensor_tensor(
                    out=Gt[:, :, 0:512].rearrange("p t (c h) -> p t c h", h=H2),
                    in0=Gt[:, :, 0:512].rearrange("p t (c h) -> p t c h", h=H2),
                    in1=Gt[:, :, 512:516][:, :, None, :].to_broadcast([128, T, C2, H2]),
                    op=ALU.mult)
                pMain = ps.tile([128, 512], F32, tag="pbig", space="PSUM")
                pS = psm[:, 96:100]
                for t in range(T):
                    nc.tensor.matmul(out=pMain[:], lhsT=Sb[:, t, :], rhs=Gt[:, t, 0:512],
                                     start=(t == 0), stop=(t == T - 1))
                    nc.tensor.matmul(out=pS, lhsT=Sb[:, t, :], rhs=Gt[:, t, 512:516],
                                     start=(t == 0), stop=(t == T - 1))
                eSl = sb.tile([128, 4], BF16, tag="eSl")
                nc.vector.tensor_tensor(out=eSl[:], in0=zg[:, 512:516],
                                        in1=arloc2[:, 4 * g:4 * g + 4], op=ALU.add)
                eSa = lrelu02(nc, sb, eSl[:], [128, 4], "eSa")
                exS = sb.tile([128, 4], BF16, tag="exS")
                nc.scalar.activation(out=exS[:], in_=eSa[:], func=ACT.Exp)
                selfc = sb.tile([128, 512], BF16, tag="selfc")
                nc.vector.tensor_tensor(
                    out=selfc[:].rearrange("p (c h) -> p c h", h=H2),
                    in0=zg[:, 0:512].rearrange("p (c h) -> p c h", h=H2),
                    in1=exS[:, None, :].to_broadcast([128, C2, H2]), op=ALU.mult)
                nc.vector.tensor_tensor(out=selfc[:], in0=pMain[:], in1=selfc[:], op=ALU.add)
                sS = sb.tile([128, 4], F32, tag="sS")
                nc.vector.tensor_tensor(out=sS[:], in0=pS, in1=exS[:], op=ALU.add)
                rS = sb.tile([128, 4], F32, tag="rS")
                nc.vector.reciprocal(out=rS[:], in_=sS[:])
                nc.vector.tensor_tensor(
                    out=selfc[:].rearrange("p (c h) -> p c h", h=H2),
                    in0=selfc[:].rearrange("p (c h) -> p c h", h=H2),
                    in1=rS[:, None, :].to_broadcast([128, C2, H2]), op=ALU.mult)
                nc.vector.tensor_tensor(out=selfc[:], in0=selfc[:], in1=C['b2t'][:], op=ALU.add)
                h2 = ln_elu(nc, sb, selfc, C['g2t'], C['be2t'], 512, epsc, "h2")
                z3p = pzp.tile([128, 20], F32, tag="pz", space="PSUM")
                for k in range(4):
                    hT = sb.tile([128, 128], BF16, tag="hT")
                    nc.sync.dma_start_transpose(out=hT[:], in_=h2[:, k * 128:(k + 1) * 128])
                    nc.tensor.matmul(out=z3p[:], lhsT=hT[:],
                                     rhs=C['W3s'][:, k * 20:(k + 1) * 20],
                                     start=(k == 0), stop=(k == 3))
                zs3 = sb.tile([128, 17], BF16, tag="zs3")
                nc.vector.tensor_copy(out=zs3[:], in_=z3p[:, 0:17])
                nc.vector.tensor_copy(out=arloc3[:, g:g + 1], in_=z3p[:, 17:18])
                nc.sync.dma_start(out=rec3_sh[r0:r0 + 128, 0:17], in_=zs3[:])
                if g in (26, 48):
                    for k in ((0, 1) if g == 26 else (2, 3)):
                        a, b = int(RB[k]), int(RB[k + 1])
                        nc.gpsimd.collective_compute(
                            "AllGather", ALU.bypass, replica_groups=[list(range(NC))],
                            ins=[rec3_sh[a:b, :]], outs=[rec3f[k][:, :]])

            # ================= L3 + pooling =================
            pPool = pacc.tile([128, 17], F32, tag="pPool", space="PSUM")
            for g in range(NG):
                r0 = g * 128
                T, o = Tg[g], int(toff[g])
                ig = sb.tile([128, T * 8], I16, tag="ig")
                nc.sync.dma_start(out=ig[:], in_=t_idx[:, o * 8:(o + T) * 8])
                Sb = sgb.tile([128, T, 128], BF16, tag="Sb")
                nc.sync.dma_start(
                    out=Sb[:, :, :],
                    in_=t_Su8[:, o * 128:(o + T) * 128].rearrange("p (t j) -> p t j", j=128))
                STb = sgb.tile([128, T, 128], BF16, tag="STb")
                nc.sync.dma_start(
                    out=STb[:, :, :],
                    in_=t_STu8[:, o * 128:(o + T) * 128].rearrange("p (t j) -> p t j", j=128))
                zg3 = sb.tile([128, 17], BF16, tag="zg3")
                nc.sync.dma_start(out=zg3[:], in_=rec3_sh[r0:r0 + 128, 0:17])
                Gt3 = gb.tile([128, T, REC3], BF16, tag="Gt3")
                tcur = 0
                for k in range(4):
                    Tk = Tc[g][k]
                    if Tk:
                        nc.gpsimd.dma_gather(
                            Gt3[:, tcur:tcur + Tk, :], rec3f[k][:, :],
                            ig[:, tcur * 8:(tcur + Tk) * 8], Tk * 128, Tk * 128, REC3)
                        tcur += Tk
                psm = ps.tile([128, 512], F32, tag="psm", space="PSUM")
                pAR3 = psm[:, 0:T]
                for t in range(T):
                    nc.tensor.matmul(out=psm[:, t:t + 1], lhsT=STb[:, t, :],
                                     rhs=arloc3[:, g:g + 1], start=True, stop=True)
                eL3 = sb.tile([128, T], BF16, tag="eL3")
                nc.vector.tensor_tensor(out=eL3[:], in0=Gt3[:, :, 16], in1=pAR3, op=ALU.add)
                eA3 = lrelu02(nc, sb, eL3[:], [128, T], "eA3")
                nc.scalar.activation(out=Gt3[:, :, 16], in_=eA3[:], func=ACT.Exp)
                nc.vector.tensor_tensor(
                    out=Gt3[:, :, 0:16], in0=Gt3[:, :, 0:16],
                    in1=Gt3[:, :, 16:17].to_broadcast([128, T, 16]), op=ALU.mult)
                pM3 = ps.tile([128, 16], F32, tag="pbig", space="PSUM")
                pS3 = psm[:, 96:97]
                for t in range(T):
                    nc.tensor.matmul(out=pM3[:], lhsT=Sb[:, t, :], rhs=Gt3[:, t, 0:16],
                                     start=(t == 0), stop=(t == T - 1))
                    nc.tensor.matmul(out=pS3, lhsT=Sb[:, t, :], rhs=Gt3[:, t, 16:17],
                                     start=(t == 0), stop=(t == T - 1))
                eS3 = sb.tile([128, 1], BF16, tag="eS3")
                nc.vector.tensor_tensor(out=eS3[:], in0=zg3[:, 16:17],
                                        in1=arloc3[:, g:g + 1], op=ALU.add)
                eA3s = lrelu02(nc, sb, eS3[:], [128, 1], "eA3s")
                exS3 = sb.tile([128, 1], F32, tag="exS3")
                nc.scalar.activation(out=exS3[:], in_=eA3s[:], func=ACT.Exp)
                selfc3 = sb.tile([128, 16], BF16, tag="selfc3")
                nc.vector.tensor_scalar(out=selfc3[:], in0=zg3[:, 0:16],
                                        scalar1=exS3[:, :1], scalar2=None, op0=ALU.mult)
                nc.vector.tensor_tensor(out=selfc3[:], in0=pM3[:], in1=selfc3[:], op=ALU.add)
                sS3 = sb.tile([128, 1], F32, tag="sS3")
                nc.vector.tensor_tensor(out=sS3[:], in0=pS3, in1=exS3[:], op=ALU.add)
                rS3 = sb.tile([128, 1], F32, tag="rS3")
                nc.vector.reciprocal(out=rS3[:], in_=sS3[:])
                nc.vector.tensor_scalar(out=selfc3[:], in0=selfc3[:], scalar1=rS3[:, :1],
                                        scalar2=None, op0=ALU.mult)
                nc.vector.tensor_tensor(out=selfc3[:], in0=selfc3[:], in1=C['b3t'][:], op=ALU.add)
                OB = sb.tile([128, 50], BF16, tag="OB")
                nc.vector.tensor_tensor(
                    out=OB[:], in0=C['iotaF50'][:, 0:50],
                    in1=batv[:, g:g + 1].to_broadcast([128, 50]), op=ALU.is_equal)
                h3w = sb.tile([128, 17], BF16, tag="h3w")
                ln_elu(nc, sb, selfc3, C['g3t'], C['be3t'], 16, epsc, "h3",
                       out_ap=h3w[:, 0:16])
                nc.vector.memset(h3w[:, 16:17], 1.0)
                nc.tensor.matmul(out=pPool[:G, :], lhsT=OB[:], rhs=h3w[:],
                                 start=(g == 0), stop=(g == NG - 1))
            po = sb.tile([128, 17], F32, tag="po")
            nc.vector.tensor_copy(out=po[:G, :], in_=pPool[:G, :])
            nc.sync.dma_start(out=t_out[:, :], in_=po[:G, :])
    nc.finalize()
    return nc


# ======================= host emulation (debug) =======================
def emulate(consts, percore, meta, host):
    """Numpy mirror of the device program (f32; validates indices/layout)."""
    Tc, NTT = meta['Tc'], meta['NTT']
    Tg = [sum(r) for r in Tc]
    toff = np.concatenate([[0], np.cumsum(Tg)]).astype(int)
    Cc = {k: np.asarray(v, np.float32) for k, v in consts.items()}
    Wf2 = np.concatenate([Cc['W2s'][:, k * 520:(k + 1) * 520] for k in range(4)], 0)
    Wf3 = np.concatenate([Cc['W3s'][:, k * 20:(k + 1) * 20] for k in range(4)], 0)

    def ln(y, gt, bt):
        m = y.mean(-1, keepdims=True)
        v = ((y - m) ** 2).mean(-1, keepdims=True)
        return (y - m) / np.sqrt(v + EPS) * gt + bt

    def elu(y):
        return np.where(y > 0, y, np.exp(np.minimum(y, 0)) - 1.0)

    def lrel(x):
        return np.where(x > 0, x, 0.2 * x)

    rec2 = np.zeros((NC, NPCP, 516), np.float32)
    ar2 = np.zeros((NC, NPCP, 4), np.float32)
    for c in range(NC):
        pc = percore[c]
        cntT4 = np.asarray(pc['cntT4'], np.float32)
        ETlo = np.asarray(pc['ETlo'], np.float32)
        EThi = np.asarray(pc['EThi'], np.float32)
        for g in range(NG):
            cols = slice(g * 128, g * 128 + 128)
            Mlo = ETlo[:, cols] * cntT4[:, cols]
            Mhi = EThi[:, cols] * cntT4[:, cols]
            pO = Mlo.T @ Cc['z1w_lo'] + Mhi.T @ Cc['z1w_hi']
            s8 = Mlo.T @ Cc['ind_lo'] + Mhi.T @ Cc['ind_hi']
            y = (pO.reshape(128, C1, H1) / s8[:, None, :]).reshape(128, 512)
            y = y + Cc['b1t'][0]
            h1 = elu(ln(y, Cc['g1t'][0], Cc['be1t'][0]))
            z2 = h1 @ Wf2
            rec2[c, cols] = z2[:, 0:516]
            ar2[c, cols] = z2[:, 516:520]
    full2 = [np.zeros((CSZ, 516), np.float32) for _ in range(4)]
    for k in range(4):
        a, b = RB[k], RB[k + 1]
        for c in range(NC):
            full2[k][c * 1568:(c + 1) * 1568] = rec2[c, a:b]

    def unwrap(idx16, tbase, Tk):
        w = idx16[:16, tbase * 8:(tbase + Tk) * 8]
        return w.T.flatten().astype(np.int64)

    def layer_edges(c, pc, fulltab, arloc, zloc, Hn, Cn):
        """Returns per-core [NPCP, Hn*Cn] aggregated output (pre-bias)."""
        nzc = Hn * Cn
        out = np.zeros((NPCP, nzc), np.float32)
        idx16 = pc['idx16']
        Su8 = pc['Su8']
        STu8 = pc['STu8']
        for g in range(NG):
            T, o = Tg[g], int(toff[g])
            Gt = np.zeros((128, T, nzc + Hn), np.float32)
            tcur = 0
            for k in range(4):
                Tk = Tc[g][k]
                if Tk:
                    L = unwrap(idx16, o + tcur, Tk)
                    Gt[:, tcur:tcur + Tk] = \
                        fulltab[k][L.reshape(Tk, 128)].transpose(1, 0, 2)
                    tcur += Tk
            S = Su8[:, o * 128:(o + T) * 128].reshape(128, T, 128).astype(np.float32)
            ST = STu8[:, o * 128:(o + T) * 128].reshape(128, T, 128).astype(np.float32)
            arg = arloc[g * 128:(g + 1) * 128]          # [128, Hn]
            pAR = np.einsum('jtp,jh->pth', ST, arg)
            eL = Gt[:, :, nzc:nzc + Hn] + pAR
            EX = np.exp(lrel(eL))                        # [128, T, Hn]
            Gz = Gt[:, :, 0:nzc].reshape(128, T, Cn, Hn) * EX[:, :, None, :]
            Gz = Gz.reshape(128, T, nzc)
            pM = np.einsum('ptj,ptc->jc', S, Gz)
            pD = np.einsum('ptj,pth->jh', S, EX)
            zgz = zloc[g * 128:(g + 1) * 128]
            exS = np.exp(lrel(zgz[:, nzc:nzc + Hn] + arg))
            num = pM + (zgz[:, 0:nzc].reshape(128, Cn, Hn) * exS[:, None, :]).reshape(128, nzc)
            dden = pD + exS
            res = (num.reshape(128, Cn, Hn) / dden[:, None, :]).reshape(128, nzc)
            out[g * 128:(g + 1) * 128] = res
        return out

    rec3 = np.zeros((NC, NPCP, 17), np.float32)
    ar3 = np.zeros((NC, NPCP, 1), np.float32)
    h3s = []
    for c in range(NC):
        agg = layer_edges(c, percore[c], full2, ar2[c], rec2[c], H2, C2)
        h2r = np.zeros((NPCP, 512), np.float32)
        for g in range(NG):
            rows = slice(g * 128, g * 128 + 128)
            y = agg[rows] + Cc['b2t'][0]
            h2 = elu(ln(y, Cc['g2t'][0], Cc['be2t'][0]))
            h2r[rows] = h2
            z3 = h2 @ Wf3
            rec3[c, rows] = z3[:, 0:17]
            ar3[c, rows] = z3[:, 17:18]
    full3 = [np.zeros((CSZ, 17), np.float32) for _ in range(4)]
    for k in range(4):
        a, b = RB[k], RB[k + 1]
        for c in range(NC):
            full3[k][c * 1568:(c + 1) * 1568] = rec3[c, a:b]

    parts = np.zeros((NC, G, 17), np.float32)
    for c in range(NC):
        agg = layer_edges(c, percore[c], full3, ar3[c], rec3[c], H3, C3)
        bat = percore[c]['batchc']                       # [128, NG]
        for g in range(NG):
            rows = slice(g * 128, g * 128 + 128)
            y = agg[rows] + Cc['b3t'][0]
            h3 = elu(ln(y, Cc['g3t'][0], Cc['be3t'][0]))
            OB = (np.arange(50)[None, :] == bat[:, g][:, None]).astype(np.float32)
            h3w = np.concatenate([h3, np.ones((128, 1), np.float32)], 1)
            parts[c] += OB.T @ h3w
    tot = parts.sum(0)
    pooled = tot[:, :16] / np.maximum(tot[:, 16:17], 1.0)
    h = np.maximum(pooled @ host['fcW1'] + host['fcb1'], 0.0)
    return (h @ host['fcW2'] + host['fcb2']).astype(np.float32)


# ======================= kernel entry =======================
_CACHE = {}


def kernel(**inputs):
    consts, percore, meta, host = host_prep(inputs)
    key = tuple(tuple(r) for r in meta['Tc'])
    if key not in _CACHE:
        _CACHE[key] = build(meta)
    nc = _CACHE[key]
    in_maps = []
    for c in range(NC):
        m = dict(consts)
        m.update(percore[c])
        in_maps.append(m)
    from concourse.bass_utils import run_bass_kernel_spmd
    res = run_bass_kernel_spmd(nc, in_maps, core_ids=list(range(NC)))
    parts = np.stack([r["part"] for r in res.results])
    tot = parts.sum(axis=0)
    pooled = tot[:, :16] / np.maximum(tot[:, 16:17], 1.0)
    h = np.maximum(pooled @ host['fcW1'] + host['fcb1'], 0.0)
    return (h @ host['fcW2'] + host['fcb2']).astype(np.float32)
